# revision 1
# baseline (speedup 1.0000x reference)
"""DeformableFeatureAlignment fused Trainium2 kernel.

Sharding: data-parallel over (batch, row-half): core c handles batch c//2,
output rows 32*(c%2) .. +32. All weights replicated. The DCNv2 bilinear
gather runs on GPSIMD indirect_copy against a channels-on-partitions
coarse_up table; row rotation makes the SPMD program core-independent.

Self-contained: hardcodes shapes from the problem spec.
"""
import sys

if "/opt/trn_rl_repo" not in sys.path:
    sys.path.insert(0, "/opt/trn_rl_repo")

import numpy as np
from contextlib import ExitStack

import concourse.bass as bass
import concourse.tile as tile
from concourse import bacc, mybir
from concourse.bass_utils import run_bass_kernel_spmd

FP32 = mybir.dt.float32
FP32R = mybir.dt.float32r
BF16 = mybir.dt.bfloat16
U16 = mybir.dt.uint16
I32 = mybir.dt.int32
I16 = mybir.dt.int16
AF = mybir.ActivationFunctionType
OP = mybir.AluOpType

B, H, W, C, F = 4, 64, 64, 256, 256
DG, K, KK = 8, 3, 9
CPG = C // DG  # 32
NCORES = 8
POS = 32 * W          # 2048 positions per core (32 rows)
HALO = 34 * W         # 2176 halo positions (34 rows)
NPH = 2               # phases
PPOS = POS // NPH     # 1024 positions per phase
OMF = DG * 3 * KK     # 216


# ---------------------------------------------------------------- host prep
def _host_consts():
    """Core-independent constants."""
    om_perm = np.zeros(OMF, np.int64)
    for g in range(DG):
        for k in range(KK):
            om_perm[g * KK + k] = g * 2 * KK + 2 * k          # dy rows 0..71
            om_perm[72 + g * KK + k] = g * 2 * KK + 2 * k + 1  # dx rows 72..143
            om_perm[144 + g * KK + k] = 144 + g * KK + k       # mask rows
    ky = np.repeat(np.arange(K) - 1, K).astype(np.float32)     # k//3 - 1
    kx = np.tile(np.arange(K) - 1, K).astype(np.float32)       # k%3 - 1

    # one-hot replication lhsT: OH[t,k][row=(g,k'), p] = 1 iff k'==k and
    # g == 4t + p//32
    oh = np.zeros((2, KK, 72, 128), np.float32)
    for t in range(2):
        for k in range(KK):
            for p in range(128):
                g = 4 * t + p // 32
                oh[t, k, g * KK + k, p] = 1.0
    return om_perm, ky, kx, oh


def _prep_weights(attend_w, select_w, offset_w, om_w, om_b, dcn_w, dcn_b):
    om_perm, ky, kx, oh = _host_consts()
    w_att = (attend_w / (H * W)).astype(np.float32)            # fold GAP mean
    wp = np.einsum("co,dof->dcf", offset_w, om_w.reshape(KK, 2 * F, OMF))
    wp = wp.copy()
    wp[:, F:, :] *= 2.0                                        # coarse_up * 2
    wp = wp[:, :, om_perm].astype(np.float32)                  # [9,512,216]
    om_bp = om_b[om_perm].astype(np.float32)
    dcn_w9 = dcn_w.reshape(KK, C, F)
    import ml_dtypes
    dcn_wb = dcn_w9.astype(ml_dtypes.bfloat16)
    ohb = oh.astype(ml_dtypes.bfloat16)
    wpb = wp.astype(ml_dtypes.bfloat16)
    return dict(w_att=w_att, w_sel=select_w.astype(np.float32), wp=wp,
                wpb=wpb, om_bp=om_bp, dcn_wb=dcn_wb,
                dcn_b=dcn_b.astype(np.float32), ohb=ohb, ky=ky, kx=kx)


def _core_inputs(core, fine, coarse, wd):
    """Build the in_map for one core."""
    b, half = core // 2, core % 2
    r0 = 32 * half
    fb = np.asarray(fine[b], np.float32).reshape(H * W, C)
    # halo rows r0-1 .. r0+32 with zero pad outside the image
    halo = np.zeros((34, W, C), np.float32)
    lo, hi = r0 - 1, r0 + 33
    src_lo, src_hi = max(lo, 0), min(hi, H)
    halo[src_lo - lo:src_hi - lo] = np.asarray(fine[b], np.float32)[src_lo:src_hi]
    # coarse rows u in [0,34): true row (r0//2 - 1 + u) mod 32
    cb = np.asarray(coarse[b], np.float32)                     # [32,32,256]
    rows = (r0 // 2 - 1 + np.arange(34)) % 32
    coarse_rot = cb[rows].reshape(34 * 32, C)

    # y-pass per-partition weights [128, 8blk, 2(a,b)] for E and O slot sets
    # even-s slots se: sources crot[se], crot[se+1]; weights (0.75, 0.25)
    # (odd fine row y), except y==63 -> (1.0, 0.0)
    # odd-s slots so: sources crot[so], crot[so+1]; weights (0.25, 0.75)
    # (even fine row y), except y==0 -> (0.0, 1.0)
    we = np.zeros((128, 8, 2), np.float32)
    wo = np.zeros((128, 8, 2), np.float32)
    for blk in range(8):
        for i in range(4):
            se = blk * 4 + i                    # slot s = 2*se
            y = (r0 - 1 + 2 * se) % H           # odd fine row
            pa, pb = (1.0, 0.0) if y == H - 1 else (0.75, 0.25)
            we[32 * i:32 * (i + 1), blk, 0] = pa
            we[32 * i:32 * (i + 1), blk, 1] = pb
            so = blk * 4 + i                    # slot s = 2*so+1
            y = (r0 + 2 * so) % H               # even fine row
            pa, pb = (0.0, 1.0) if y == 0 else (0.25, 0.75)
            wo[32 * i:32 * (i + 1), blk, 0] = pa
            wo[32 * i:32 * (i + 1), blk, 1] = pb

    # YM: zero the om-halo row that lies outside the image: slot 0 (y=r0-1)
    # if r0==0 else slot 33 (y=r0+32==64)
    ym = np.ones((128, 34 * W), np.float32)
    zslot = 0 if r0 == 0 else 33
    ym[:, zslot * W:(zslot + 1) * W] = 0.0

    pos = np.arange(POS, dtype=np.float32)
    yg = r0 + pos // W
    xg = pos % W
    cy = (yg[None, :] + wd["ky"].repeat(DG).reshape(KK, DG).T.reshape(72, 1)
          ) if False else None
    # CY[r=(g,k), pos] = y_global(pos) + ky[k]; CX likewise
    kyr = np.tile(wd["ky"], DG)                                # [72] (g,k)
    kxr = np.tile(wd["kx"], DG)
    cy = (yg[None, :] + kyr[:, None]).astype(np.float32)
    cx = (xg[None, :] + kxr[:, None]).astype(np.float32)
    cr = np.full((72, 1), float(r0 - 1), np.float32)

    return {
        "fine_full": fb, "fine_halo": halo.reshape(HALO, C),
        "coarse_rot": coarse_rot,
        "w_att": wd["w_att"], "w_sel": wd["w_sel"],
        "wpb": wd["wpb"],
        "dcn_wb": wd["dcn_wb"], "dcn_b": wd["dcn_b"].reshape(F, 1),
        "ohb": wd["ohb"],
        "cy": cy, "cx": cx, "cr": cr, "ym": ym,
        "ymb": __import__("ml_dtypes") and ym.astype(
            __import__("ml_dtypes").bfloat16),
        "we": we.reshape(128, 16), "wo": wo.reshape(128, 16),
        "idt": np.eye(128, dtype=np.float32),
        "ones": np.ones((128, 1), np.float32),
    }


# ---------------------------------------------------------------- device
def _build_nc():
    nc = bacc.Bacc("TRN2", target_bir_lowering=False, debug=False)
    dt = nc.dram_tensor
    fine_full = dt("fine_full", [H * W, C], FP32, kind="ExternalInput").ap()
    fine_halo = dt("fine_halo", [HALO, C], FP32, kind="ExternalInput").ap()
    coarse_rot = dt("coarse_rot", [34 * 32, C], FP32, kind="ExternalInput").ap()
    w_att = dt("w_att", [C, C], FP32, kind="ExternalInput").ap()
    w_sel = dt("w_sel", [C, F], FP32, kind="ExternalInput").ap()
    wpb = dt("wpb", [KK, 2 * F, OMF], BF16, kind="ExternalInput").ap()
    dcn_wb = dt("dcn_wb", [KK, C, F], BF16, kind="ExternalInput").ap()
    dcn_b = dt("dcn_b", [F, 1], FP32, kind="ExternalInput").ap()
    ohb = dt("ohb", [2, KK, 72, 128], BF16, kind="ExternalInput").ap()
    cy_d = dt("cy", [72, POS], FP32, kind="ExternalInput").ap()
    cx_d = dt("cx", [72, POS], FP32, kind="ExternalInput").ap()
    cr_d = dt("cr", [72, 1], FP32, kind="ExternalInput").ap()
    ym_d = dt("ym", [128, HALO], FP32, kind="ExternalInput").ap()
    ymb_d = dt("ymb", [128, HALO], BF16, kind="ExternalInput").ap()
    we_d = dt("we", [128, 16], FP32, kind="ExternalInput").ap()
    wo_d = dt("wo", [128, 16], FP32, kind="ExternalInput").ap()
    idt_d = dt("idt", [128, 128], FP32, kind="ExternalInput").ap()
    ones_d = dt("ones", [128, 1], FP32, kind="ExternalInput").ap()
    out_d = dt("out", [POS, C], FP32, kind="ExternalOutput").ap()
    idx_scr = dt("idx_scr", [2, 4, 2, 16, KK, 2, 128], I16).ap()

    with TileCtx(nc) as tc, ExitStack() as ctx:
        v, s, pe, gp = nc.vector, nc.scalar, nc.tensor, nc.gpsimd
        pool = lambda name, bufs: ctx.enter_context(tc.tile_pool(name=name, bufs=bufs))

        cst = pool("cst", 1)
        idt = cst.tile([128, 128], FP32); nc.sync.dma_start(idt[:], idt_d)
        ones = cst.tile([128, 1], FP32); nc.sync.dma_start(ones[:], ones_d)
        crt = cst.tile([72, 1], FP32); nc.sync.dma_start(crt[:], cr_d)
        dcnbt = cst.tile([128, 2], FP32)
        nc.sync.dma_start(dcnbt[:], dcn_b.rearrange("(c p) one -> p (c one)", c=2))
        oht = cst.tile([72, 2 * KK * 128], BF16)
        nc.sync.dma_start(
            oht[:].rearrange("r (t k p) -> r t k p", t=2, k=KK),
            ohb.rearrange("t k r p -> r t k p"))

        # persistent: E tables (paired bf16), fcal, AL, weights+indices
        big = pool("big", 1)
        ET = [big.tile([128, H * W * 2], BF16, name=f"et{i}", tag=f"et{i}")
              for i in range(2)]
        fcal = [big.tile([128, HALO], FP32, name=f"fcal{i}", tag=f"fcal{i}")
                for i in range(2)]
        AL = [big.tile([128, POS], FP32, name=f"al{i}", tag=f"al{i}")
              for i in range(2)]
        W0i = big.tile([72, 2 * POS], BF16)
        W1i = big.tile([72, 2 * POS], BF16)
        IUS = [big.tile([72, POS], I16, name=f"ius{_i}", tag=f"ius{_i}")
               for _i in range(2)]
        IW = [big.tile([128, KK * 2 * 128], I16, name=f"iw{_i}")
              for _i in range(2)]

        with tc.tile_pool(name="dyxp", bufs=1) as dyx:
            DY = dyx.tile([72, POS], FP32)
            DX = dyx.tile([72, POS], FP32)
            MS = dyx.tile([72, POS], FP32)

            with tc.tile_pool(name="stage1", bufs=1) as st1, \
                 tc.tile_pool(name="ld", bufs=6) as ld, \
                 tc.tile_pool(name="tp_ps", bufs=2, space="PSUM") as tp_ps:
                fht = [st1.tile([128, HALO], BF16, name=f"fht{i}",
                                tag=f"fht{i}") for i in range(2)]
                wsel = st1.tile([128, 2 * F], FP32)
                wselb = st1.tile([128, 2 * F], BF16)
                ymt = st1.tile([128, HALO], FP32)
                nc.sync.dma_start(ymt[:], ym_d)
                wet = st1.tile([128, 16], FP32); nc.sync.dma_start(wet[:], we_d)
                wot = st1.tile([128, 16], FP32); nc.sync.dma_start(wot[:], wo_d)
                gap = st1.tile([128, 2], FP32)
                sct = st1.tile([128, 2], FP32)

                # ---- GAP via ones-matmul over fine_full
                with tc.tile_pool(name="gap_ps", bufs=1, space="PSUM") as gpp:
                    gap_ps = [gpp.tile([128, 1], FP32, name=f"gps{i}")
                              for i in range(2)]
                    for i in range(32):
                        t = ld.tile([128, C], FP32, tag="fln")
                        nc.sync.dma_start(t[:],
                                          fine_full[128 * i:128 * (i + 1), :])
                        for h_ in range(2):
                            pe.matmul(gap_ps[h_][:],
                                      t[:, 128 * h_:128 * (h_ + 1)],
                                      ones[:], start=(i == 0), stop=(i == 31))
                    for h_ in range(2):
                        s.copy(gap[:, h_:h_ + 1], gap_ps[h_][:])

                # ---- attn -> wselb
                with tc.tile_pool(name="att", bufs=2) as attp, \
                     tc.tile_pool(name="att_ps", bufs=1, space="PSUM") as atpp:
                    for fc in range(2):
                        aps = atpp.tile([128, 1], FP32, tag="aps")
                        for cc in range(2):
                            wt = attp.tile([128, 128], FP32, tag="watt")
                            nc.sync.dma_start(
                                wt[:], w_att[128 * cc:128 * (cc + 1),
                                             128 * fc:128 * (fc + 1)])
                            pe.matmul(aps[:], wt[:], gap[:, cc:cc + 1],
                                      start=(cc == 0), stop=(cc == 1))
                        s.activation(sct[:, fc:fc + 1], aps[:], AF.Sigmoid)
                    v.tensor_scalar(sct[:], sct[:], 1.0, None, OP.add)
                    for cc in range(2):
                        wt = attp.tile([128, F], FP32, tag="wselld")
                        nc.sync.dma_start(wt[:],
                                          w_sel[128 * cc:128 * (cc + 1), :])
                        v.tensor_scalar(wsel[:, F * cc:F * (cc + 1)], wt[:],
                                        sct[:, cc:cc + 1], None, OP.mult)
                    v.tensor_copy(wselb[:], wsel[:])

                # ---- transpose fine_halo -> fht (bf16)
                for i in range(17):
                    t = ld.tile([128, C], FP32, tag="fhl")
                    nc.sync.dma_start(t[:], fine_halo[128 * i:128 * (i + 1), :])
                    for cc in range(2):
                        ps = tp_ps.tile([128, 128], FP32, tag="tp")
                        pe.transpose(ps[:], t[:, 128 * cc:128 * (cc + 1)],
                                     idt[:])
                        s.copy(fht[cc][:, 128 * i:128 * (i + 1)], ps[:])

                # ---- coarse: y-pass, transpose, x-pass -> ET[...,0]
                with tc.tile_pool(name="yeyo", bufs=1) as yeyo:
                    ye = yeyo.tile([128, 8 * C], FP32)
                    yo = yeyo.tile([128, 8 * C], FP32)
                    yev = ye[:].rearrange("p (i c) -> p i c", i=8)
                    yov = yo[:].rearrange("p (i c) -> p i c", i=8)
                    with tc.tile_pool(name="crot", bufs=1) as crotp:
                        crA = crotp.tile([128, 8 * C], FP32)
                        crB = crotp.tile([128, 9 * C], FP32)
                        crAv = crA[:].rearrange("p (i c) -> p i c", i=8)
                        crBv = crB[:].rearrange("p (i c) -> p i c", i=9)
                        nc.sync.dma_start(
                            crAv,
                            coarse_rot[:1024].rearrange("(i p) c -> p i c",
                                                        p=128))
                        nc.sync.dma_start(
                            crBv[:, 0:8, :],
                            coarse_rot[32:1056].rearrange("(i p) c -> p i c",
                                                          p=128))
                        nc.sync.dma_start(crBv[0:32, 8, :],
                                          coarse_rot[1056:1088])
                        for blk in range(8):
                            for (dstv, wtile) in ((yev, wet), (yov, wot)):
                                tb = crotp.tile([128, C], FP32, tag="yt")
                                v.tensor_scalar(
                                    tb[:], crBv[:, blk, :],
                                    wtile[:, 2 * blk + 1:2 * blk + 2],
                                    None, OP.mult)
                                v.scalar_tensor_tensor(
                                    dstv[:, blk, :], crAv[:, blk, :],
                                    wtile[:, 2 * blk:2 * blk + 1], tb[:],
                                    OP.mult, OP.add)
                    with tc.tile_pool(name="cupf", bufs=1) as cupf:
                        cmid = [cupf.tile([128, H * 32], FP32, name=f"cmid{i}",
                                          tag=f"cmid{i}") for i in range(2)]
                        for ysrc, par in ((yev, 0), (yov, 1)):
                            for i in range(8):
                                for cc in range(2):
                                    ps = tp_ps.tile([128, 128], FP32, tag="tp")
                                    pe.transpose(
                                        ps[:],
                                        ysrc[:, i, 128 * cc:128 * (cc + 1)],
                                        idt[:])
                                    dst = cmid[cc][:].rearrange(
                                        "p (s2 two x) -> p s2 two x",
                                        two=2, x=32)
                                    s.copy(dst[:, 4 * i:4 * (i + 1), par, :],
                                           ps[:])
                        for cc in range(2):
                            mid = cmid[cc][:].rearrange("p (s x) -> p s x",
                                                        s=H)
                            ev = ET[cc][:].rearrange("p (s x two) -> p s x two",
                                                     s=H, two=2)
                            out3 = ev[:, :, :, 0]
                            a = cupf.tile([128, H * 32], FP32, tag="xq")
                            v.tensor_scalar(a[:], cmid[cc][:], 0.25, None,
                                            OP.mult)
                            a3 = a[:].rearrange("p (s x) -> p s x", s=H)
                            v.scalar_tensor_tensor(out3[:, :, 2:64:2],
                                                   mid[:, :, 1:], 0.75,
                                                   a3[:, :, 0:31],
                                                   OP.mult, OP.add)
                            v.tensor_copy(out3[:, :, 0:1], mid[:, :, 0:1])
                            v.scalar_tensor_tensor(out3[:, :, 1:63:2],
                                                   mid[:, :, 0:31], 0.75,
                                                   a3[:, :, 1:32],
                                                   OP.mult, OP.add)
                            v.tensor_copy(out3[:, :, 63:64], mid[:, :, 31:32])
                            # pair slot: E[:, q, 1] = E[:, q+1, 0]
                            ef = ET[cc][:].rearrange("p (q two) -> p q two",
                                                     two=2)
                            v.tensor_copy(ef[:, 0:H * W - 1, 1],
                                          ef[:, 1:H * W, 0])
                            v.tensor_copy(ef[:, H * W - 1:H * W, 1],
                                          ef[:, H * W - 1:H * W, 0])

                # ---- fine_cal = wselb.T @ fht
                with tc.tile_pool(name="fc_ps", bufs=2, space="PSUM") as fc_ps:
                    for fc_ in range(2):
                        for pblk in range(5):
                            n0 = 512 * pblk
                            n1 = min(n0 + 512, HALO)
                            ps = fc_ps.tile([128, 512], FP32, tag="fc")
                            for cc in range(2):
                                pe.matmul(ps[:, :n1 - n0],
                                          wselb[:, F * cc + 128 * fc_:
                                                F * cc + 128 * fc_ + 128],
                                          fht[cc][:, n0:n1],
                                          start=(cc == 0), stop=(cc == 1))
                            s.copy(fcal[fc_][:, n0:n1], ps[:, :n1 - n0])

                # ---- om conv on padded inputs
                with tc.tile_pool(name="ppad", bufs=1) as ppad, \
                     tc.tile_pool(name="om_ps", bufs=2, space="PSUM") as om_ps:
                    ptf = [ppad.tile([128, 34 * 66], BF16, name=f"ptf{i}",
                                     tag=f"ptf{i}") for i in range(2)]
                    ptc = [ppad.tile([128, 34 * 66], BF16, name=f"ptc{i}",
                                     tag=f"ptc{i}") for i in range(2)]
                    for i in range(2):
                        gp.memset(ptf[i][:], 0.0)
                        v.tensor_tensor(
                            ptf[i][:].rearrange("p (y x) -> p y x",
                                                y=34)[:, :, 1:65],
                            fcal[i][:].rearrange("p (y x) -> p y x", y=34),
                            ymt[:].rearrange("p (y x) -> p y x", y=34),
                            OP.mult)
                        gp.memset(ptc[i][:], 0.0)
                        ev = ET[i][:].rearrange("p (s x two) -> p s x two",
                                                s=H, two=2)
                        v.tensor_tensor(
                            ptc[i][:].rearrange("p (y x) -> p y x",
                                                y=34)[:, :, 1:65],
                            ev[:, 0:34, :, 0],
                            ymt[:].rearrange("p (y x) -> p y x", y=34),
                            OP.mult)
                    womt = ppad.tile([128, KK * 4 * OMF], BF16)
                    nc.sync.dma_start(
                        womt[:].rearrange("p (k c f) -> p k c f", k=KK, c=4),
                        wpb.rearrange("k (c p) f -> p k c f", p=128))
                    womv = womt[:].rearrange("p (k c f) -> p k c f", k=KK, c=4)
                    for pblk in range(4):
                        psD = om_ps.tile([72, 512], FP32, tag="omD")
                        psX = om_ps.tile([72, 512], FP32, tag="omX")
                        psM = om_ps.tile([72, 512], FP32, tag="omM")
                        for k in range(KK):
                            dy, dx = k // 3 - 1, k % 3 - 1
                            for cc in range(4):
                                pt_ = (ptf[cc] if cc < 2 else ptc[cc - 2])
                                rv = pt_[:].rearrange("p (y x) -> p y x",
                                                      y=34)[
                                    :, 1 + 8 * pblk + dy:9 + 8 * pblk + dy,
                                    1 + dx:65 + dx]
                                wv = womv[:, k, cc, :]
                                first = (k == 0 and cc == 0)
                                last = (k == KK - 1 and cc == 3)
                                pe.matmul(psD[:], wv[:, 0:72], rv,
                                          start=first, stop=last)
                                pe.matmul(psX[:], wv[:, 72:144], rv,
                                          start=first, stop=last)
                                pe.matmul(psM[:], wv[:, 144:216], rv,
                                          start=first, stop=last)
                        c0 = 512 * pblk
                        s.copy(DY[:, c0:c0 + 512], psD[:])
                        s.copy(DX[:, c0:c0 + 512], psX[:])
                        s.activation(MS[:, c0:c0 + 512], psM[:], AF.Sigmoid)

            # ---- index & weight math (stage1 freed)
            with tc.tile_pool(name="mth", bufs=1) as mth, \
                 tc.tile_pool(name="cycx", bufs=4) as cycx:
                CH = 512
                tn = lambda nm: mth.tile([72, CH], FP32, name=nm, tag=nm)
                ti = mth.tile([72, CH], I32)
                tf, tg, XP, FL, FR, X1 = (tn(x) for x in
                                          ("tf", "tg", "XP", "FL", "FR", "X1"))
                R0c, R1c, VA, VB, WY0, WY1, SSc, A0, A1 = (
                    tn(x) for x in ("R0c", "R1c", "VA", "VB", "WY0", "WY1",
                                    "SSc", "A0", "A1"))

                def floor_(dst, srcp):
                    v.tensor_copy(ti[:], srcp[:])
                    v.tensor_copy(tf[:], ti[:])
                    v.tensor_tensor(tg[:], tf[:], srcp[:], OP.is_gt)
                    v.tensor_tensor(dst[:], tf[:], tg[:], OP.subtract)

                W0v = W0i[:].rearrange("p (n t) -> p n t", t=2)
                W1v = W1i[:].rearrange("p (n t) -> p n t", t=2)
                for ch in range(POS // CH):
                    c0 = CH * ch
                    sl = slice(c0, c0 + CH)
                    cyt = cycx.tile([72, CH], FP32, tag="cyt")
                    nc.sync.dma_start(cyt[:], cy_d[:, sl])
                    cxt = cycx.tile([72, CH], FP32, tag="cxt")
                    nc.sync.dma_start(cxt[:], cx_d[:, sl])
                    # y side
                    v.tensor_tensor(XP[:], DY[:, sl], cyt[:], OP.add)
                    floor_(FL, XP)
                    v.tensor_tensor(FR[:], XP[:], FL[:], OP.subtract)
                    v.tensor_scalar(X1[:], FL[:], 1.0, None, OP.add)
                    v.tensor_scalar(VA[:], FL[:], 0.0, None, OP.is_ge)
                    v.tensor_scalar(tg[:], FL[:], 63.0, None, OP.is_le)
                    v.tensor_tensor(VA[:], VA[:], tg[:], OP.mult)
                    v.tensor_scalar(VB[:], X1[:], 0.0, None, OP.is_ge)
                    v.tensor_scalar(tg[:], X1[:], 63.0, None, OP.is_le)
                    v.tensor_tensor(VB[:], VB[:], tg[:], OP.mult)
                    v.tensor_scalar(tf[:], FR[:], -1.0, 1.0, OP.mult, OP.add)
                    v.tensor_tensor(tf[:], tf[:], VA[:], OP.mult)
                    v.tensor_tensor(WY0[:], tf[:], MS[:, sl], OP.mult)
                    v.tensor_tensor(tf[:], FR[:], VB[:], OP.mult)
                    v.tensor_tensor(WY1[:], tf[:], MS[:, sl], OP.mult)
                    v.tensor_scalar(R0c[:], FL[:], 0.0, 63.0, OP.max, OP.min)
                    v.tensor_scalar(R1c[:], X1[:], 0.0, 63.0, OP.max, OP.min)
                    # x side
                    v.tensor_tensor(XP[:], DX[:, sl], cxt[:], OP.add)
                    floor_(FL, XP)
                    v.tensor_tensor(FR[:], XP[:], FL[:], OP.subtract)
                    v.tensor_scalar(X1[:], FL[:], 1.0, None, OP.add)
                    v.tensor_scalar(VA[:], FL[:], 0.0, None, OP.is_ge)
                    v.tensor_scalar(tg[:], FL[:], 63.0, None, OP.is_le)
                    v.tensor_tensor(VA[:], VA[:], tg[:], OP.mult)
                    v.tensor_scalar(VB[:], X1[:], 0.0, None, OP.is_ge)
                    v.tensor_scalar(tg[:], X1[:], 63.0, None, OP.is_le)
                    v.tensor_tensor(VB[:], VB[:], tg[:], OP.mult)
                    v.tensor_scalar(tf[:], FR[:], -1.0, 1.0, OP.mult, OP.add)
                    v.tensor_tensor(A0[:], tf[:], VA[:], OP.mult)
                    v.tensor_tensor(A1[:], FR[:], VB[:], OP.mult)
                    v.tensor_scalar(SSc[:], FL[:], 0.0, 62.0, OP.max, OP.min)
                    for sidx in range(2):
                        if sidx == 0:
                            v.tensor_tensor(tf[:], SSc[:], FL[:], OP.is_equal)
                            v.tensor_tensor(tg[:], SSc[:], X1[:], OP.is_equal)
                        else:
                            v.tensor_scalar(VA[:], SSc[:], 1.0, None, OP.add)
                            v.tensor_tensor(tf[:], VA[:], FL[:], OP.is_equal)
                            v.tensor_tensor(tg[:], VA[:], X1[:], OP.is_equal)
                        v.tensor_tensor(tf[:], tf[:], A0[:], OP.mult)
                        v.tensor_tensor(tg[:], tg[:], A1[:], OP.mult)
                        v.tensor_tensor(tf[:], tf[:], tg[:], OP.add)
                        v.tensor_tensor(W0v[:, sl, sidx], WY0[:], tf[:],
                                        OP.mult)
                        v.tensor_tensor(W1v[:, sl, sidx], WY1[:], tf[:],
                                        OP.mult)
                    # rotated slot index -> IUS (wrapped free order)
                    for ri, rr in ((0, R0c), (1, R1c)):
                        v.tensor_scalar(tf[:], rr[:], crt[:, 0:1], None,
                                        OP.subtract)
                        v.tensor_scalar(tg[:], tf[:], 0.0, None, OP.is_lt)
                        v.scalar_tensor_tensor(tf[:], tg[:], 64.0, tf[:],
                                               OP.mult, OP.add)
                        v.tensor_scalar(tg[:], tf[:], 64.0, None, OP.is_ge)
                        v.scalar_tensor_tensor(tf[:], tg[:], -64.0, tf[:],
                                               OP.mult, OP.add)
                        v.tensor_scalar(tf[:], tf[:], 64.0, None, OP.mult)
                        v.tensor_tensor(tf[:], tf[:], SSc[:], OP.add)
                        ncol = CH // 16
                        dstv = IUS[ri][:].rearrange(
                            "p (s c) -> p s c", s=16)[
                            :, :, ncol * ch:ncol * (ch + 1)].rearrange(
                            "p s c -> p c s")
                        v.tensor_copy(dstv,
                                      tf[:].rearrange("p (c s) -> p c s",
                                                      s=16))

        # ---- idx bounce: duplicated wrapped layout
        for t_ in range(2):
            for d_ in range(2):
                for g_ in range(4):
                    for r_ in range(2):
                        nc.sync.dma_start(
                            idx_scr[t_, g_, d_, :, :, r_, :].rearrange(
                                "p k col -> k p col"),
                            IUS[r_][36 * t_ + 9 * g_:36 * t_ + 9 * (g_ + 1), :]
                            .rearrange("k (p col) -> k p col", p=16))
        for t_ in range(2):
            nc.sync.dma_start(
                IW[t_][:],
                idx_scr[t_].rearrange("g d p k r col -> (g d p) (k r col)"))

        # ---- main loop
        with tc.tile_pool(name="dcnw", bufs=1) as dcnw_p, \
             tc.tile_pool(name="gat", bufs=3) as gat, \
             tc.tile_pool(name="wrep", bufs=2) as wrep, \
             tc.tile_pool(name="rep_ps", bufs=3, space="PSUM") as rep_ps, \
             tc.tile_pool(name="dcn_ps", bufs=1, space="PSUM") as dcn_ps, \
             tc.tile_pool(name="val", bufs=2) as val_p:
            dwt = dcnw_p.tile([128, 2 * KK * F], BF16)
            nc.sync.dma_start(
                dwt[:].rearrange("p (k f) -> p k f", k=2 * KK),
                dcn_wb.rearrange("k (c p) f -> p (k c) f", p=128))
            dwv = dwt[:].rearrange("p (k f) -> p k f", k=2 * KK)

            ohv = oht[:].rearrange("r (t k p) -> r t k p", t=2, k=KK)
            for ph in range(NPH):
                p0 = PPOS * ph
                dps = [dcn_ps.tile([128, 512], FP32, name=f"dcn{ph}_{i}",
                                   tag=f"dcn{i}") for i in range(4)]
                for t_ in range(2):
                    for k in range(KK):
                        wr = []
                        for r_, wi in ((0, W0i), (1, W1i)):
                            wre = wrep.tile([128, 2 * PPOS], BF16,
                                            tag=f"wr{r_}")
                            for ns in range(4):
                                ps = rep_ps.tile([128, 512], FP32, tag="rep")
                                pe.matmul(ps[:], ohv[:, t_, k, :],
                                          wi[:, 2 * p0 + 512 * ns:
                                             2 * p0 + 512 * (ns + 1)],
                                          start=True, stop=True)
                                s.copy(wre[:, 512 * ns:512 * (ns + 1)], ps[:])
                            wr.append(wre)
                        gt = []
                        for r_ in range(2):
                            g_ = gat.tile([128, PPOS, 2], BF16, tag=f"g{r_}")
                            gp.ap_gather(
                                g_[:],
                                ET[t_][:].rearrange("p (q two) -> p q two",
                                                    two=2),
                                IW[t_][:].rearrange(
                                    "p (k r col) -> p k r col", k=KK, r=2)[
                                    :, k, r_, 64 * ph:64 * (ph + 1)],
                                channels=128, num_elems=H * W, d=2,
                                num_idxs=PPOS)
                            gt.append(g_)
                        P0 = val_p.tile([128, 2 * PPOS], BF16, tag="p0")
                        P1 = val_p.tile([128, 2 * PPOS], BF16, tag="p1")
                        val = val_p.tile([128, PPOS], BF16, tag="val")
                        g0f = gt[0][:].rearrange("p n t -> p (n t)")
                        g1f = gt[1][:].rearrange("p n t -> p (n t)")
                        v.tensor_tensor(P0[:], g0f, wr[0][:], OP.mult)
                        v.tensor_tensor(P1[:], g1f, wr[1][:], OP.mult)
                        v.tensor_tensor(P0[:], P0[:], P1[:], OP.add)
                        p0v = P0[:].rearrange("p (n t) -> p n t", t=2)
                        v.tensor_tensor(val[:], p0v[:, :, 0], p0v[:, :, 1],
                                        OP.add)
                        first = (t_ == 0 and k == 0)
                        last = (t_ == 1 and k == KK - 1)
                        for fc_ in range(2):
                            for ns in range(2):
                                pe.matmul(
                                    dps[2 * fc_ + ns][:],
                                    dwv[:, 2 * k + t_,
                                        128 * fc_:128 * (fc_ + 1)],
                                    val[:, 512 * ns:512 * (ns + 1)],
                                    start=first, stop=last)
                for fc_ in range(2):
                    for ns in range(2):
                        s.activation(
                            AL[fc_][:, p0 + 512 * ns:p0 + 512 * (ns + 1)],
                            dps[2 * fc_ + ns][:], AF.Relu,
                            bias=dcnbt[:, fc_:fc_ + 1])

        # ---- final add + transpose + out
        with tc.tile_pool(name="fin_ps", bufs=2, space="PSUM") as fin_ps, \
             tc.tile_pool(name="fin_sb", bufs=2) as fin_sb:
            for fc_ in range(2):
                v.tensor_tensor(AL[fc_][:], AL[fc_][:],
                                fcal[fc_][:, W:W + POS], OP.add)
            for i in range(16):
                ot = fin_sb.tile([128, C], FP32, tag="ot")
                for fc_ in range(2):
                    ps = fin_ps.tile([128, 128], FP32, tag="fin")
                    pe.transpose(ps[:], AL[fc_][:, 128 * i:128 * (i + 1)],
                                 idt[:])
                    s.copy(ot[:, 128 * fc_:128 * (fc_ + 1)], ps[:])
                nc.sync.dma_start(out_d[128 * i:128 * (i + 1), :], ot[:])

    nc.compile()
    return nc


TileCtx = tile.TileContext
_NC_CACHE = None


def _get_nc():
    global _NC_CACHE
    if _NC_CACHE is None:
        _NC_CACHE = _build_nc()
    return _NC_CACHE


def kernel(fine, coarse, attend_w, select_w, offset_w, om_w, om_b, dcn_w,
           dcn_b, _trace=False, _trace_kwargs=None):
    wd = _prep_weights(np.asarray(attend_w), np.asarray(select_w),
                       np.asarray(offset_w), np.asarray(om_w),
                       np.asarray(om_b), np.asarray(dcn_w), np.asarray(dcn_b))
    in_maps = [_core_inputs(c, np.asarray(fine), np.asarray(coarse), wd)
               for c in range(NCORES)]
    nc = _get_nc()
    kw = {}
    if _trace:
        import concourse.bass_utils as _bu
        _bu.upload_artifacts = lambda d: d
        kw = dict(trace=True, trace_kwargs=_trace_kwargs or {})
    res = run_bass_kernel_spmd(nc, in_maps, list(range(NCORES)), **kw)
    out = np.zeros((B, H, W, C), np.float32)
    for c in range(NCORES):
        b, half = c // 2, c % 2
        out[b, 32 * half:32 * half + 32] = res.results[c]["out"].reshape(32, W, C)
    if _trace:
        kernel._last_exec_ns = res.exec_time_ns
    return out



# revision 7
# speedup vs baseline: 2.2624x; 2.2624x over previous
"""DeformableFeatureAlignment fused Trainium2 kernel.

Sharding: data-parallel over (batch, row-half): core c handles batch c//2,
output rows 32*(c%2) .. +32. All weights replicated. The DCNv2 bilinear
gather runs on GPSIMD ap_gather against a channels-on-partitions
coarse_up patch table; row rotation makes the SPMD program core-independent.

Gather layout: one [128, 4096, 8] bf16 table where partition p=(g,j)
holds channels (32g+j, 32g+j+16) and each 8-wide block is the full 2x2
bilinear patch for both channels, so one index per (pos, tap, group)
fetches all 4 corners x 2 channels. Host permutes coarse channels (and
the matching om/dcn weight rows) so the pair lands on one partition.

Self-contained: hardcodes shapes from the problem spec.
"""
import sys

if "/opt/trn_rl_repo" not in sys.path:
    sys.path.insert(0, "/opt/trn_rl_repo")

import numpy as np
from contextlib import ExitStack

import concourse.bass as bass
import concourse.tile as tile
from concourse import bacc, mybir
from concourse.bass_utils import run_bass_kernel_spmd

FP32 = mybir.dt.float32
BF16 = mybir.dt.bfloat16
I32 = mybir.dt.int32
I16 = mybir.dt.int16
AF = mybir.ActivationFunctionType
OP = mybir.AluOpType

B, H, W, C, F = 4, 64, 64, 256, 256
DG, K, KK = 8, 3, 9
CPG = C // DG  # 32
NCORES = 8
POS = 32 * W          # 2048 positions per core (32 rows)
HALO = 34 * W         # 2176 halo positions (34 rows)
NPH = 2               # phases
PPOS = POS // NPH     # 1024 positions per phase
OMF = DG * 3 * KK     # 216

# channel permutation: partition p (0..127) holds channels
# (PERM[p], PERM[p]+16); both stay inside deformable group p//16.
PERM = np.array([32 * (p // 16) + p % 16 for p in range(128)])
PERM256 = np.concatenate([PERM, PERM + 16])


# ---------------------------------------------------------------- host prep
def _host_consts():
    """Core-independent constants."""
    om_perm = np.zeros(OMF, np.int64)
    for g in range(DG):
        for k in range(KK):
            om_perm[g * KK + k] = g * 2 * KK + 2 * k          # dy rows 0..71
            om_perm[72 + g * KK + k] = g * 2 * KK + 2 * k + 1  # dx rows 72..143
            om_perm[144 + g * KK + k] = 144 + g * KK + k       # mask rows
    ky = np.repeat(np.arange(K) - 1, K).astype(np.float32)     # k//3 - 1
    kx = np.tile(np.arange(K) - 1, K).astype(np.float32)       # k%3 - 1

    # one-hot replication lhsT: OH[k][row=(g,k'), p] = 1 iff k'==k and
    # g == p//16
    oh = np.zeros((KK, 72, 128), np.float32)
    for k in range(KK):
        for p in range(128):
            oh[k, (p // 16) * KK + k, p] = 1.0
    return om_perm, ky, kx, oh


def _prep_weights(attend_w, select_w, offset_w, om_w, om_b, dcn_w, dcn_b):
    om_perm, ky, kx, oh = _host_consts()
    w_att = (attend_w / (H * W)).astype(np.float32)            # fold GAP mean
    wp = np.einsum("co,dof->dcf", offset_w, om_w.reshape(KK, 2 * F, OMF))
    wp = wp.copy()
    wp[:, F:, :] *= 2.0                                        # coarse_up * 2
    wp[:, F:, :] = wp[:, F + PERM256, :]                       # permuted coarse
    wp = wp[:, :, om_perm].astype(np.float32)                  # [9,512,216]
    dcn_w9 = dcn_w.reshape(KK, C, F)
    # dwAB[k, h, p, f]: dcn weight rows in the packed-channel order
    dwAB = np.stack([dcn_w9[:, PERM, :], dcn_w9[:, PERM + 16, :]], 1)
    import ml_dtypes
    dcn_wb = dwAB.astype(ml_dtypes.bfloat16)                   # [KK,2,128,F]
    ohb = oh.astype(ml_dtypes.bfloat16)
    wpb = wp.astype(ml_dtypes.bfloat16)
    return dict(w_att=w_att, w_sel=select_w.astype(np.float32),
                wpb=wpb, dcn_wb=dcn_wb,
                dcn_b=dcn_b.astype(np.float32), ohb=ohb, ky=ky, kx=kx)


def _core_inputs(core, fine, coarse, wd):
    """Build the in_map for one core."""
    b, half = core // 2, core % 2
    r0 = 32 * half
    fb = np.asarray(fine[b], np.float32).reshape(H * W, C)
    # halo rows r0-1 .. r0+32 with zero pad outside the image
    halo = np.zeros((34, W, C), np.float32)
    lo, hi = r0 - 1, r0 + 33
    src_lo, src_hi = max(lo, 0), min(hi, H)
    halo[src_lo - lo:src_hi - lo] = np.asarray(fine[b], np.float32)[src_lo:src_hi]
    # coarse rows u in [0,34): true row (r0//2 - 1 + u) mod 32; channels
    # permuted so partition pairing matches the packed table.
    cb = np.asarray(coarse[b], np.float32)[:, :, PERM256]      # [32,32,256]
    rows = (r0 // 2 - 1 + np.arange(34)) % 32
    coarse_rot = cb[rows].reshape(34 * 32, C)

    # y-pass per-partition weights [128, 8blk, 2(a,b)] for E and O slot sets
    we = np.zeros((128, 8, 2), np.float32)
    wo = np.zeros((128, 8, 2), np.float32)
    for blk in range(8):
        for i in range(4):
            se = blk * 4 + i                    # slot s = 2*se
            y = (r0 - 1 + 2 * se) % H           # odd fine row
            pa, pb = (1.0, 0.0) if y == H - 1 else (0.75, 0.25)
            we[32 * i:32 * (i + 1), blk, 0] = pa
            we[32 * i:32 * (i + 1), blk, 1] = pb
            so = blk * 4 + i                    # slot s = 2*so+1
            y = (r0 + 2 * so) % H               # even fine row
            pa, pb = (0.0, 1.0) if y == 0 else (0.25, 0.75)
            wo[32 * i:32 * (i + 1), blk, 0] = pa
            wo[32 * i:32 * (i + 1), blk, 1] = pb

    # YM: zero the om-halo row that lies outside the image: slot 0 (y=r0-1)
    # if r0==0 else slot 33 (y=r0+32==64)
    import ml_dtypes
    ym = np.ones((128, 34 * W), ml_dtypes.bfloat16)
    zslot = 0 if r0 == 0 else 33
    ym[:, zslot * W:(zslot + 1) * W] = 0.0

    pos = np.arange(POS, dtype=np.float32)
    yg = r0 + pos // W
    xg = pos % W
    # CY[r=(g,k), pos] = y_global(pos) + ky[k]; CX likewise
    kyr = np.tile(wd["ky"], DG)                                # [72] (g,k)
    kxr = np.tile(wd["kx"], DG)
    cy = (yg[None, :] + kyr[:, None]).astype(np.float32)
    cx = (xg[None, :] + kxr[:, None]).astype(np.float32)
    cr = np.full((72, 1), float(r0 - 1), np.float32)

    return {
        "fine_full": fb, "fine_halo": halo.reshape(HALO, C),
        "coarse_rot": coarse_rot,
        "w_att": wd["w_att"], "w_sel": wd["w_sel"],
        "wpb": wd["wpb"],
        "dcn_wb": wd["dcn_wb"], "dcn_b": wd["dcn_b"].reshape(F, 1),
        "ohb": wd["ohb"],
        "cy": cy, "cx": cx, "cr": cr, "ym": ym,
        "we": we.reshape(128, 16), "wo": wo.reshape(128, 16),
        "idt": np.eye(128, dtype=np.float32),
        "ones": np.ones((128, 1), np.float32),
    }


# ---------------------------------------------------------------- device
def _build_nc():
    nc = bacc.Bacc("TRN2", target_bir_lowering=False, debug=False)
    dt = nc.dram_tensor
    fine_full = dt("fine_full", [H * W, C], FP32, kind="ExternalInput").ap()
    fine_halo = dt("fine_halo", [HALO, C], FP32, kind="ExternalInput").ap()
    coarse_rot = dt("coarse_rot", [34 * 32, C], FP32, kind="ExternalInput").ap()
    w_att = dt("w_att", [C, C], FP32, kind="ExternalInput").ap()
    w_sel = dt("w_sel", [C, F], FP32, kind="ExternalInput").ap()
    wpb = dt("wpb", [KK, 2 * F, OMF], BF16, kind="ExternalInput").ap()
    dcn_wb = dt("dcn_wb", [KK, 2, 128, F], BF16, kind="ExternalInput").ap()
    dcn_b = dt("dcn_b", [F, 1], FP32, kind="ExternalInput").ap()
    ohb = dt("ohb", [KK, 72, 128], BF16, kind="ExternalInput").ap()
    cy_d = dt("cy", [72, POS], FP32, kind="ExternalInput").ap()
    cx_d = dt("cx", [72, POS], FP32, kind="ExternalInput").ap()
    cr_d = dt("cr", [72, 1], FP32, kind="ExternalInput").ap()
    ym_d = dt("ym", [128, HALO], BF16, kind="ExternalInput").ap()
    we_d = dt("we", [128, 16], FP32, kind="ExternalInput").ap()
    wo_d = dt("wo", [128, 16], FP32, kind="ExternalInput").ap()
    idt_d = dt("idt", [128, 128], FP32, kind="ExternalInput").ap()
    ones_d = dt("ones", [128, 1], FP32, kind="ExternalInput").ap()
    out_d = dt("out", [POS, C], FP32, kind="ExternalOutput").ap()
    idx_scr = dt("idx_scr", [8, 16, KK, 128], I16).ap()

    with TileCtx(nc) as tc, ExitStack() as ctx:
        v, s, pe, gp = nc.vector, nc.scalar, nc.tensor, nc.gpsimd
        pool = lambda name, bufs: ctx.enter_context(tc.tile_pool(name=name, bufs=bufs))

        cst = pool("cst", 1)
        idt = cst.tile([128, 128], FP32); nc.sync.dma_start(idt[:], idt_d)
        ones = cst.tile([128, 1], FP32); nc.sync.dma_start(ones[:], ones_d)
        crt = cst.tile([72, 1], FP32); nc.sync.dma_start(crt[:], cr_d)
        dcnbt = cst.tile([128, 2], FP32)
        nc.sync.dma_start(dcnbt[:], dcn_b.rearrange("(c p) one -> p (c one)", c=2))
        oht = cst.tile([72, KK * 128], BF16)
        nc.sync.dma_start(
            oht[:].rearrange("r (k p) -> r k p", k=KK),
            ohb.rearrange("k r p -> r k p"))

        # persistent: CT patch table, fcal, AL, weights+indices
        big = pool("big", 1)
        CT = big.tile([128, H * W * 8], BF16, name="ct", tag="ct")
        fcal = [big.tile([128, HALO], FP32, name=f"fcal{i}", tag=f"fcal{i}")
                for i in range(2)]
        AL = [big.tile([128, POS], FP32, name=f"al{i}", tag=f"al{i}")
              for i in range(2)]
        W4i = big.tile([72, 4 * POS], BF16)
        IUS = big.tile([72, POS], I16, name="ius", tag="ius")
        IW = big.tile([128, KK * 128], I16, name="iw")

        with tc.tile_pool(name="dyxp", bufs=1) as dyx:
            DY = dyx.tile([72, POS], FP32)
            DX = dyx.tile([72, POS], FP32)
            MS = dyx.tile([72, POS], FP32)

            with tc.tile_pool(name="stage1", bufs=1) as st1, \
                 tc.tile_pool(name="ld", bufs=4) as ld, \
                 tc.tile_pool(name="tp_ps", bufs=2, space="PSUM") as tp_ps:
                fht = [st1.tile([128, HALO], BF16, name=f"fht{i}",
                                tag=f"fht{i}") for i in range(2)]
                wselb = st1.tile([128, 2 * F], BF16)
                ymt = st1.tile([128, HALO], BF16)
                nc.sync.dma_start(ymt[:], ym_d)
                wet = st1.tile([128, 16], FP32); nc.sync.dma_start(wet[:], we_d)
                wot = st1.tile([128, 16], FP32); nc.sync.dma_start(wot[:], wo_d)
                gap = st1.tile([128, 2], FP32)
                sct = st1.tile([128, 2], FP32)

                # ---- GAP via ones-matmul over fine_full
                with tc.tile_pool(name="gap_ps", bufs=1, space="PSUM") as gpp:
                    gap_ps = [gpp.tile([128, 1], FP32, name=f"gps{i}")
                              for i in range(2)]
                    for i in range(32):
                        t = ld.tile([128, C], FP32, tag="fln")
                        nc.sync.dma_start(t[:],
                                          fine_full[128 * i:128 * (i + 1), :])
                        for h_ in range(2):
                            pe.matmul(gap_ps[h_][:],
                                      t[:, 128 * h_:128 * (h_ + 1)],
                                      ones[:], start=(i == 0), stop=(i == 31))
                    for h_ in range(2):
                        s.copy(gap[:, h_:h_ + 1], gap_ps[h_][:])

                # ---- attn -> wselb
                with tc.tile_pool(name="att", bufs=2) as attp, \
                     tc.tile_pool(name="att_ps", bufs=1, space="PSUM") as atpp:
                    for fc in range(2):
                        aps = atpp.tile([128, 1], FP32, tag="aps")
                        for cc in range(2):
                            wt = attp.tile([128, 128], FP32, tag="watt")
                            nc.sync.dma_start(
                                wt[:], w_att[128 * cc:128 * (cc + 1),
                                             128 * fc:128 * (fc + 1)])
                            pe.matmul(aps[:], wt[:], gap[:, cc:cc + 1],
                                      start=(cc == 0), stop=(cc == 1))
                        s.activation(sct[:, fc:fc + 1], aps[:], AF.Sigmoid)
                    v.tensor_scalar(sct[:], sct[:], 1.0, None, OP.add)
                    for cc in range(2):
                        wt = attp.tile([128, F], FP32, tag="wselld")
                        nc.sync.dma_start(wt[:],
                                          w_sel[128 * cc:128 * (cc + 1), :])
                        v.tensor_scalar(wselb[:, F * cc:F * (cc + 1)], wt[:],
                                        sct[:, cc:cc + 1], None, OP.mult)

                # ---- transpose fine_halo -> fht (bf16)
                for i in range(17):
                    t = ld.tile([128, C], FP32, tag="fhl")
                    nc.sync.dma_start(t[:], fine_halo[128 * i:128 * (i + 1), :])
                    for cc in range(2):
                        ps = tp_ps.tile([128, 128], FP32, tag="tp")
                        pe.transpose(ps[:], t[:, 128 * cc:128 * (cc + 1)],
                                     idt[:])
                        s.copy(fht[cc][:, 128 * i:128 * (i + 1)], ps[:])

                # ---- coarse: y-pass, transpose, x-pass -> CT patch table
                # CT layout: [p, q=(s*64+x), e=8]:
                #   e=4*cc+0: v(s, x)    e=4*cc+1: v(s, x+1)
                #   e=4*cc+2: v(s+1, x)  e=4*cc+3: v(s+1, x+1)
                # for channel half cc (cc=0: PERM[p], cc=1: PERM[p]+16);
                # s+1 wraps to slot 0 (rotation is cyclic in global rows).
                with tc.tile_pool(name="yeyo", bufs=1) as yeyo:
                    ye = yeyo.tile([128, 8 * C], FP32)
                    yo = yeyo.tile([128, 8 * C], FP32)
                    yev = ye[:].rearrange("p (i c) -> p i c", i=8)
                    yov = yo[:].rearrange("p (i c) -> p i c", i=8)
                    with tc.tile_pool(name="crot", bufs=1) as crotp:
                        crA = crotp.tile([128, 8 * C], FP32)
                        crB = crotp.tile([128, 9 * C], FP32)
                        crAv = crA[:].rearrange("p (i c) -> p i c", i=8)
                        crBv = crB[:].rearrange("p (i c) -> p i c", i=9)
                        nc.sync.dma_start(
                            crAv,
                            coarse_rot[:1024].rearrange("(i p) c -> p i c",
                                                        p=128))
                        nc.sync.dma_start(
                            crBv[:, 0:8, :],
                            coarse_rot[32:1056].rearrange("(i p) c -> p i c",
                                                          p=128))
                        nc.sync.dma_start(crBv[0:32, 8, :],
                                          coarse_rot[1056:1088])
                        for blk in range(8):
                            for (dstv, wtile) in ((yev, wet), (yov, wot)):
                                tb = crotp.tile([128, C], FP32, tag="yt")
                                v.tensor_scalar(
                                    tb[:], crBv[:, blk, :],
                                    wtile[:, 2 * blk + 1:2 * blk + 2],
                                    None, OP.mult)
                                v.scalar_tensor_tensor(
                                    dstv[:, blk, :], crAv[:, blk, :],
                                    wtile[:, 2 * blk:2 * blk + 1], tb[:],
                                    OP.mult, OP.add)
                    with tc.tile_pool(name="cupf", bufs=1) as cupf:
                        cmid = [cupf.tile([128, H * 32], BF16, name=f"cmid{i}",
                                          tag=f"cmid{i}") for i in range(2)]
                        for ysrc, par in ((yev, 0), (yov, 1)):
                            for i in range(8):
                                for cc in range(2):
                                    ps = tp_ps.tile([128, 128], FP32, tag="tp")
                                    pe.transpose(
                                        ps[:],
                                        ysrc[:, i, 128 * cc:128 * (cc + 1)],
                                        idt[:])
                                    dst = cmid[cc][:].rearrange(
                                        "p (s2 two x) -> p s2 two x",
                                        two=2, x=32)
                                    s.copy(dst[:, 4 * i:4 * (i + 1), par, :],
                                           ps[:])
                        ctq = CT[:].rearrange("p (q e) -> p q e", e=8)
                        ct3 = CT[:].rearrange("p (s x e) -> p s x e",
                                              s=H, e=8)
                        for cc in range(2):
                            mid = cmid[cc][:].rearrange("p (s x) -> p s x",
                                                        s=H)
                            out3 = ct3[:, :, :, 4 * cc]
                            a = cupf.tile([128, H * 32], BF16, tag="xq")
                            v.tensor_scalar(a[:], cmid[cc][:], 0.25, None,
                                            OP.mult)
                            a3 = a[:].rearrange("p (s x) -> p s x", s=H)
                            v.scalar_tensor_tensor(out3[:, :, 2:64:2],
                                                   mid[:, :, 1:], 0.75,
                                                   a3[:, :, 0:31],
                                                   OP.mult, OP.add)
                            v.tensor_copy(out3[:, :, 0:1], mid[:, :, 0:1])
                            v.scalar_tensor_tensor(out3[:, :, 1:63:2],
                                                   mid[:, :, 0:31], 0.75,
                                                   a3[:, :, 1:32],
                                                   OP.mult, OP.add)
                            v.tensor_copy(out3[:, :, 63:64], mid[:, :, 31:32])
                            # x-pair slot: e+1 at q = e+0 at q+1
                            v.tensor_copy(ctq[:, 0:H * W - 1, 4 * cc + 1],
                                          ctq[:, 1:H * W, 4 * cc])
                            v.tensor_copy(ctq[:, H * W - 1:H * W, 4 * cc + 1],
                                          ctq[:, H * W - 1:H * W, 4 * cc])
                            # next-row slots: e+2:e+4 at (s,x) = e:e+2 at
                            # (s+1,x); s=63 wraps to slot 0
                            v.tensor_copy(ct3[:, 0:H - 1, :, 4 * cc + 2:4 * cc + 4],
                                          ct3[:, 1:H, :, 4 * cc:4 * cc + 2])
                            v.tensor_copy(ct3[:, H - 1:H, :, 4 * cc + 2:4 * cc + 4],
                                          ct3[:, 0:1, :, 4 * cc:4 * cc + 2])

                # ---- fine_cal = wselb.T @ fht
                with tc.tile_pool(name="fc_ps", bufs=2, space="PSUM") as fc_ps:
                    for fc_ in range(2):
                        for pblk in range(5):
                            n0 = 512 * pblk
                            n1 = min(n0 + 512, HALO)
                            ps = fc_ps.tile([128, 512], FP32, tag="fc")
                            for cc in range(2):
                                pe.matmul(ps[:, :n1 - n0],
                                          wselb[:, F * cc + 128 * fc_:
                                                F * cc + 128 * fc_ + 128],
                                          fht[cc][:, n0:n1],
                                          start=(cc == 0), stop=(cc == 1))
                            s.copy(fcal[fc_][:, n0:n1], ps[:, :n1 - n0])

                # ---- om conv on padded inputs
                with tc.tile_pool(name="ppad", bufs=1) as ppad, \
                     tc.tile_pool(name="om_ps", bufs=2, space="PSUM") as om_ps:
                    ptf = [ppad.tile([128, 34 * 66], BF16, name=f"ptf{i}",
                                     tag=f"ptf{i}") for i in range(2)]
                    ptc = [ppad.tile([128, 34 * 66], BF16, name=f"ptc{i}",
                                     tag=f"ptc{i}") for i in range(2)]
                    for i in range(2):
                        gp.memset(ptf[i][:], 0.0)
                        v.tensor_tensor(
                            ptf[i][:].rearrange("p (y x) -> p y x",
                                                y=34)[:, :, 1:65],
                            fcal[i][:].rearrange("p (y x) -> p y x", y=34),
                            ymt[:].rearrange("p (y x) -> p y x", y=34),
                            OP.mult)
                        gp.memset(ptc[i][:], 0.0)
                        ct3 = CT[:].rearrange("p (s x e) -> p s x e",
                                              s=H, e=8)
                        v.tensor_tensor(
                            ptc[i][:].rearrange("p (y x) -> p y x",
                                                y=34)[:, :, 1:65],
                            ct3[:, 0:34, :, 4 * i],
                            ymt[:].rearrange("p (y x) -> p y x", y=34),
                            OP.mult)
                    womt = ppad.tile([128, KK * 4 * OMF], BF16)
                    nc.sync.dma_start(
                        womt[:].rearrange("p (k c f) -> p k c f", k=KK, c=4),
                        wpb.rearrange("k (c p) f -> p k c f", p=128))
                    womv = womt[:].rearrange("p (k c f) -> p k c f", k=KK, c=4)
                    for pblk in range(4):
                        psD = om_ps.tile([72, 512], FP32, tag="omD")
                        psX = om_ps.tile([72, 512], FP32, tag="omX")
                        psM = om_ps.tile([72, 512], FP32, tag="omM")
                        for k in range(KK):
                            dy, dx = k // 3 - 1, k % 3 - 1
                            for cc in range(4):
                                pt_ = (ptf[cc] if cc < 2 else ptc[cc - 2])
                                rv = pt_[:].rearrange("p (y x) -> p y x",
                                                      y=34)[
                                    :, 1 + 8 * pblk + dy:9 + 8 * pblk + dy,
                                    1 + dx:65 + dx]
                                wv = womv[:, k, cc, :]
                                first = (k == 0 and cc == 0)
                                last = (k == KK - 1 and cc == 3)
                                pe.matmul(psD[:], wv[:, 0:72], rv,
                                          start=first, stop=last)
                                pe.matmul(psX[:], wv[:, 72:144], rv,
                                          start=first, stop=last)
                                pe.matmul(psM[:], wv[:, 144:216], rv,
                                          start=first, stop=last)
                        c0 = 512 * pblk
                        s.copy(DY[:, c0:c0 + 512], psD[:])
                        s.copy(DX[:, c0:c0 + 512], psX[:])
                        s.activation(MS[:, c0:c0 + 512], psM[:], AF.Sigmoid)

            # ---- index & weight math (stage1 freed)
            # One index per (pos, tap, group): q = rot(SY)*64 + SX with
            # SY = clip(floor(yp), 0, 62), SX = clip(floor(xp), 0, 62);
            # 4 corner weights with clamp-equality redistribution.
            with tc.tile_pool(name="mth", bufs=1) as mth, \
                 tc.tile_pool(name="cycx", bufs=4) as cycx:
                CH = 512
                tn = lambda nm: mth.tile([72, CH], FP32, name=nm, tag=nm)
                ti = mth.tile([72, CH], I32)
                tf, tg, XP, FL, FR, X1 = (tn(x) for x in
                                          ("tf", "tg", "XP", "FL", "FR", "X1"))
                VA, VB, WY0, WY1, WR0, WR1, SSc, A0, A1, WX0, WX1 = (
                    tn(x) for x in ("VA", "VB", "WY0", "WY1", "WR0", "WR1",
                                    "SSc", "A0", "A1", "WX0", "WX1"))

                def floor_(dst, srcp):
                    v.tensor_copy(ti[:], srcp[:])
                    v.tensor_copy(tf[:], ti[:])
                    v.tensor_tensor(tg[:], tf[:], srcp[:], OP.is_gt)
                    v.tensor_tensor(dst[:], tf[:], tg[:], OP.subtract)

                W4v = W4i[:].rearrange("p (n e) -> p n e", e=4)
                for ch in range(POS // CH):
                    c0 = CH * ch
                    sl = slice(c0, c0 + CH)
                    cyt = cycx.tile([72, CH], FP32, tag="cyt")
                    nc.sync.dma_start(cyt[:], cy_d[:, sl])
                    cxt = cycx.tile([72, CH], FP32, tag="cxt")
                    nc.sync.dma_start(cxt[:], cx_d[:, sl])
                    # y side
                    v.tensor_tensor(XP[:], DY[:, sl], cyt[:], OP.add)
                    floor_(FL, XP)
                    v.tensor_tensor(FR[:], XP[:], FL[:], OP.subtract)
                    v.tensor_scalar(X1[:], FL[:], 1.0, None, OP.add)
                    v.tensor_scalar(VA[:], FL[:], 0.0, None, OP.is_ge)
                    v.tensor_scalar(tg[:], FL[:], 63.0, None, OP.is_le)
                    v.tensor_tensor(VA[:], VA[:], tg[:], OP.mult)
                    v.tensor_scalar(VB[:], X1[:], 0.0, None, OP.is_ge)
                    v.tensor_scalar(tg[:], X1[:], 63.0, None, OP.is_le)
                    v.tensor_tensor(VB[:], VB[:], tg[:], OP.mult)
                    v.tensor_scalar(tf[:], FR[:], -1.0, 1.0, OP.mult, OP.add)
                    v.tensor_tensor(tf[:], tf[:], VA[:], OP.mult)
                    v.tensor_tensor(WY0[:], tf[:], MS[:, sl], OP.mult)
                    v.tensor_tensor(tf[:], FR[:], VB[:], OP.mult)
                    v.tensor_tensor(WY1[:], tf[:], MS[:, sl], OP.mult)
                    # SY = clip(FL, 0, 62); row weights via equality masks
                    v.tensor_scalar(SSc[:], FL[:], 0.0, 62.0, OP.max, OP.min)
                    v.tensor_tensor(tf[:], SSc[:], FL[:], OP.is_equal)
                    v.tensor_tensor(tg[:], SSc[:], X1[:], OP.is_equal)
                    v.tensor_tensor(tf[:], tf[:], WY0[:], OP.mult)
                    v.tensor_tensor(tg[:], tg[:], WY1[:], OP.mult)
                    v.tensor_tensor(WR0[:], tf[:], tg[:], OP.add)
                    v.tensor_scalar(VA[:], SSc[:], 1.0, None, OP.add)
                    v.tensor_tensor(tf[:], VA[:], FL[:], OP.is_equal)
                    v.tensor_tensor(tg[:], VA[:], X1[:], OP.is_equal)
                    v.tensor_tensor(tf[:], tf[:], WY0[:], OP.mult)
                    v.tensor_tensor(tg[:], tg[:], WY1[:], OP.mult)
                    v.tensor_tensor(WR1[:], tf[:], tg[:], OP.add)
                    # rotated row slot: rot = SY - crt (+64 if negative);
                    # SY<=62 and crt>=-1 so rot<64 always
                    v.tensor_scalar(tf[:], SSc[:], crt[:, 0:1], None,
                                    OP.subtract)
                    v.tensor_scalar(tg[:], tf[:], 0.0, None, OP.is_lt)
                    v.scalar_tensor_tensor(tf[:], tg[:], 64.0, tf[:],
                                           OP.mult, OP.add)
                    v.tensor_scalar(WY0[:], tf[:], 64.0, None, OP.mult)
                    # x side
                    v.tensor_tensor(XP[:], DX[:, sl], cxt[:], OP.add)
                    floor_(FL, XP)
                    v.tensor_tensor(FR[:], XP[:], FL[:], OP.subtract)
                    v.tensor_scalar(X1[:], FL[:], 1.0, None, OP.add)
                    v.tensor_scalar(VA[:], FL[:], 0.0, None, OP.is_ge)
                    v.tensor_scalar(tg[:], FL[:], 63.0, None, OP.is_le)
                    v.tensor_tensor(VA[:], VA[:], tg[:], OP.mult)
                    v.tensor_scalar(VB[:], X1[:], 0.0, None, OP.is_ge)
                    v.tensor_scalar(tg[:], X1[:], 63.0, None, OP.is_le)
                    v.tensor_tensor(VB[:], VB[:], tg[:], OP.mult)
                    v.tensor_scalar(tf[:], FR[:], -1.0, 1.0, OP.mult, OP.add)
                    v.tensor_tensor(A0[:], tf[:], VA[:], OP.mult)
                    v.tensor_tensor(A1[:], FR[:], VB[:], OP.mult)
                    v.tensor_scalar(SSc[:], FL[:], 0.0, 62.0, OP.max, OP.min)
                    v.tensor_tensor(tf[:], SSc[:], FL[:], OP.is_equal)
                    v.tensor_tensor(tg[:], SSc[:], X1[:], OP.is_equal)
                    v.tensor_tensor(tf[:], tf[:], A0[:], OP.mult)
                    v.tensor_tensor(tg[:], tg[:], A1[:], OP.mult)
                    v.tensor_tensor(WX0[:], tf[:], tg[:], OP.add)
                    v.tensor_scalar(VA[:], SSc[:], 1.0, None, OP.add)
                    v.tensor_tensor(tf[:], VA[:], FL[:], OP.is_equal)
                    v.tensor_tensor(tg[:], VA[:], X1[:], OP.is_equal)
                    v.tensor_tensor(tf[:], tf[:], A0[:], OP.mult)
                    v.tensor_tensor(tg[:], tg[:], A1[:], OP.mult)
                    v.tensor_tensor(WX1[:], tf[:], tg[:], OP.add)
                    # q = rot*64 + SX -> IUS (wrapped free order)
                    v.tensor_tensor(tf[:], WY0[:], SSc[:], OP.add)
                    ncol = CH // 16
                    dstv = IUS[:].rearrange(
                        "p (s c) -> p s c", s=16)[
                        :, :, ncol * ch:ncol * (ch + 1)].rearrange(
                        "p s c -> p c s")
                    v.tensor_copy(dstv,
                                  tf[:].rearrange("p (c s) -> p c s",
                                                  s=16))
                    # corner weights -> W4i
                    v.tensor_tensor(W4v[:, sl, 0], WR0[:], WX0[:], OP.mult)
                    v.tensor_tensor(W4v[:, sl, 1], WR0[:], WX1[:], OP.mult)
                    v.tensor_tensor(W4v[:, sl, 2], WR1[:], WX0[:], OP.mult)
                    v.tensor_tensor(W4v[:, sl, 3], WR1[:], WX1[:], OP.mult)

        # ---- idx bounce: per-group wrapped layout
        for g_ in range(8):
            nc.sync.dma_start(
                idx_scr[g_].rearrange("p k col -> k p col"),
                IUS[9 * g_:9 * (g_ + 1), :]
                .rearrange("k (p col) -> k p col", p=16))
        nc.sync.dma_start(
            IW[:],
            idx_scr.rearrange("g p k col -> (g p) (k col)"))

        # ---- main loop
        with tc.tile_pool(name="dcnw", bufs=1) as dcnw_p, \
             tc.tile_pool(name="gat", bufs=2) as gat, \
             tc.tile_pool(name="wrep", bufs=2) as wrep, \
             tc.tile_pool(name="rep_ps", bufs=2, space="PSUM") as rep_ps, \
             tc.tile_pool(name="dcn_ps", bufs=1, space="PSUM") as dcn_ps, \
             tc.tile_pool(name="prod", bufs=1) as prod_p, \
             tc.tile_pool(name="val", bufs=2) as val_p:
            dwt = dcnw_p.tile([128, KK * 2 * F], BF16)
            nc.sync.dma_start(
                dwt[:].rearrange("p (k h f) -> p k h f", k=KK, h=2),
                dcn_wb.rearrange("k h p f -> p k h f"))
            dwv = dwt[:].rearrange("p (k h f) -> p k h f", k=KK, h=2)
            ctv = CT[:].rearrange("p (q e) -> p q e", e=8)

            def emit_rep(ph, k):
                """replicated 4-corner weights [128, PPOS, 4] for (ph, k)."""
                wre = wrep.tile([128, 4 * PPOS], BF16, tag="wr")
                for ns in range(8):
                    ps = rep_ps.tile([128, 512], FP32, tag="rep")
                    pe.matmul(ps[:], oht[:, 128 * k:128 * (k + 1)],
                              W4i[:, 4096 * ph + 512 * ns:
                                  4096 * ph + 512 * (ns + 1)],
                              start=True, stop=True)
                    s.copy(wre[:, 512 * ns:512 * (ns + 1)], ps[:])
                return wre

            wre_next = emit_rep(0, 0)
            for ph in range(NPH):
                p0 = PPOS * ph
                dps = [dcn_ps.tile([128, 512], FP32, name=f"dcn{ph}_{i}",
                                   tag=f"dcn{i}") for i in range(4)]
                for k in range(KK):
                    wre = wre_next
                    g_ = gat.tile([128, PPOS, 8], BF16, tag="g")
                    gp.ap_gather(
                        g_[:], ctv,
                        IW[:, 128 * k + 64 * ph:128 * k + 64 * ph + 64],
                        channels=128, num_elems=H * W, d=8,
                        num_idxs=PPOS)
                    # pre-emit next iteration's replication so PE keeps
                    # ahead of the serial gather stream
                    if not (ph == NPH - 1 and k == KK - 1):
                        nk = (k + 1) % KK
                        nph = ph + 1 if nk == 0 else ph
                        wre_next = emit_rep(nph, nk)
                    PA = prod_p.tile([128, PPOS, 4], BF16, tag="pa")
                    PB = prod_p.tile([128, PPOS, 4], BF16, tag="pb")
                    T1 = prod_p.tile([128, PPOS, 2], BF16, tag="t1")
                    valA = val_p.tile([128, PPOS], BF16, tag="valA")
                    valB = val_p.tile([128, PPOS], BF16, tag="valB")
                    wr4 = wre[:].rearrange("p (n e) -> p n e", e=4)
                    v.tensor_tensor(PA[:], g_[:, :, 0:4], wr4, OP.mult)
                    v.tensor_tensor(PB[:], g_[:, :, 4:8], wr4, OP.mult)
                    v.tensor_tensor(T1[:], PA[:, :, 0:2], PA[:, :, 2:4],
                                    OP.add)
                    v.tensor_tensor(valA[:], T1[:, :, 0], T1[:, :, 1],
                                    OP.add)
                    v.tensor_tensor(T1[:], PB[:, :, 0:2], PB[:, :, 2:4],
                                    OP.add)
                    v.tensor_tensor(valB[:], T1[:, :, 0], T1[:, :, 1],
                                    OP.add)
                    first = (k == 0)
                    last = (k == KK - 1)
                    for h_, vv in ((0, valA), (1, valB)):
                        for fc_ in range(2):
                            for ns in range(2):
                                pe.matmul(
                                    dps[2 * fc_ + ns][:],
                                    dwv[:, k, h_,
                                        128 * fc_:128 * (fc_ + 1)],
                                    vv[:, 512 * ns:512 * (ns + 1)],
                                    start=(first and h_ == 0),
                                    stop=(last and h_ == 1))
                for fc_ in range(2):
                    for ns in range(2):
                        s.activation(
                            AL[fc_][:, p0 + 512 * ns:p0 + 512 * (ns + 1)],
                            dps[2 * fc_ + ns][:], AF.Relu,
                            bias=dcnbt[:, fc_:fc_ + 1])

        # ---- final add + transpose + out
        with tc.tile_pool(name="fin_ps", bufs=2, space="PSUM") as fin_ps, \
             tc.tile_pool(name="fin_sb", bufs=2) as fin_sb:
            for fc_ in range(2):
                v.tensor_tensor(AL[fc_][:], AL[fc_][:],
                                fcal[fc_][:, W:W + POS], OP.add)
            for i in range(16):
                ot = fin_sb.tile([128, C], FP32, tag="ot")
                for fc_ in range(2):
                    ps = fin_ps.tile([128, 128], FP32, tag="fin")
                    pe.transpose(ps[:], AL[fc_][:, 128 * i:128 * (i + 1)],
                                 idt[:])
                    s.copy(ot[:, 128 * fc_:128 * (fc_ + 1)], ps[:])
                nc.sync.dma_start(out_d[128 * i:128 * (i + 1), :], ot[:])

    nc.compile()
    return nc


TileCtx = tile.TileContext
_NC_CACHE = None


def _get_nc():
    global _NC_CACHE
    if _NC_CACHE is None:
        _NC_CACHE = _build_nc()
    return _NC_CACHE


def kernel(fine, coarse, attend_w, select_w, offset_w, om_w, om_b, dcn_w,
           dcn_b, _trace=False, _trace_kwargs=None):
    wd = _prep_weights(np.asarray(attend_w), np.asarray(select_w),
                       np.asarray(offset_w), np.asarray(om_w),
                       np.asarray(om_b), np.asarray(dcn_w), np.asarray(dcn_b))
    in_maps = [_core_inputs(c, np.asarray(fine), np.asarray(coarse), wd)
               for c in range(NCORES)]
    nc = _get_nc()
    kw = {}
    if _trace:
        import concourse.bass_utils as _bu
        _bu.upload_artifacts = lambda d: d
        kw = dict(trace=True, trace_kwargs=_trace_kwargs or {})
    res = run_bass_kernel_spmd(nc, in_maps, list(range(NCORES)), **kw)
    out = np.zeros((B, H, W, C), np.float32)
    for c in range(NCORES):
        b, half = c // 2, c % 2
        out[b, 32 * half:32 * half + 32] = res.results[c]["out"].reshape(32, W, C)
    if _trace:
        kernel._last_exec_ns = res.exec_time_ns
    return out


# revision 10
# speedup vs baseline: 2.4219x; 1.0705x over previous
"""DeformableFeatureAlignment fused Trainium2 kernel.

Sharding: data-parallel over (batch, row-half): core c handles batch c//2,
output rows 32*(c%2) .. +32. All weights replicated. The DCNv2 bilinear
gather runs on GPSIMD ap_gather against a channels-on-partitions
coarse_up patch table; row rotation makes the SPMD program core-independent.

Gather layout: one [128, 4096, 8] bf16 table where partition p=(g,j)
holds channels (32g+j, 32g+j+16) and each 8-wide block is the full 2x2
bilinear patch for both channels, so one index per (pos, tap, group)
fetches all 4 corners x 2 channels. Host permutes coarse channels (and
the matching om/dcn weight rows) so the pair lands on one partition.

Self-contained: hardcodes shapes from the problem spec.
"""
import sys

if "/opt/trn_rl_repo" not in sys.path:
    sys.path.insert(0, "/opt/trn_rl_repo")

import numpy as np
from contextlib import ExitStack

import concourse.bass as bass
import concourse.tile as tile
from concourse import bacc, mybir
from concourse.bass_utils import run_bass_kernel_spmd

FP32 = mybir.dt.float32
BF16 = mybir.dt.bfloat16
I32 = mybir.dt.int32
I16 = mybir.dt.int16
AF = mybir.ActivationFunctionType
OP = mybir.AluOpType

B, H, W, C, F = 4, 64, 64, 256, 256
DG, K, KK = 8, 3, 9
CPG = C // DG  # 32
NCORES = 8
POS = 32 * W          # 2048 positions per core (32 rows)
HALO = 34 * W         # 2176 halo positions (34 rows)
NPH = 2               # phases
PPOS = POS // NPH     # 1024 positions per phase
OMF = DG * 3 * KK     # 216

# channel permutation: partition p (0..127) holds channels
# (PERM[p], PERM[p]+16); both stay inside deformable group p//16.
PERM = np.array([32 * (p // 16) + p % 16 for p in range(128)])
PERM256 = np.concatenate([PERM, PERM + 16])


# ---------------------------------------------------------------- host prep
def _host_consts():
    """Core-independent constants."""
    om_perm = np.zeros(OMF, np.int64)
    for g in range(DG):
        for k in range(KK):
            om_perm[g * KK + k] = g * 2 * KK + 2 * k          # dy rows 0..71
            om_perm[72 + g * KK + k] = g * 2 * KK + 2 * k + 1  # dx rows 72..143
            om_perm[144 + g * KK + k] = 144 + g * KK + k       # mask rows
    ky = np.repeat(np.arange(K) - 1, K).astype(np.float32)     # k//3 - 1
    kx = np.tile(np.arange(K) - 1, K).astype(np.float32)       # k%3 - 1

    # one-hot replication lhsT: OH[k][row=(g,k'), p] = 1 iff k'==k and
    # g == p//16
    oh = np.zeros((KK, 72, 128), np.float32)
    for k in range(KK):
        for p in range(128):
            oh[k, (p // 16) * KK + k, p] = 1.0
    return om_perm, ky, kx, oh


def _prep_weights(attend_w, select_w, offset_w, om_w, om_b, dcn_w, dcn_b):
    om_perm, ky, kx, oh = _host_consts()
    w_att = (attend_w / (H * W)).astype(np.float32)            # fold GAP mean
    wp = np.einsum("co,dof->dcf", offset_w, om_w.reshape(KK, 2 * F, OMF))
    wp = wp.copy()
    wp[:, F:, :] *= 2.0                                        # coarse_up * 2
    wp[:, F:, :] = wp[:, F + PERM256, :]                       # permuted coarse
    wp = wp[:, :, om_perm].astype(np.float32)                  # [9,512,216]
    dcn_w9 = dcn_w.reshape(KK, C, F)
    # dwAB[k, h, p, f]: dcn weight rows in the packed-channel order
    dwAB = np.stack([dcn_w9[:, PERM, :], dcn_w9[:, PERM + 16, :]], 1)
    import ml_dtypes
    dcn_wb = dwAB.astype(ml_dtypes.bfloat16)                   # [KK,2,128,F]
    ohb = oh.astype(ml_dtypes.bfloat16)
    wpb = wp.astype(ml_dtypes.bfloat16)
    return dict(w_att=w_att, w_sel=select_w.astype(np.float32),
                wpb=wpb, dcn_wb=dcn_wb,
                dcn_b=dcn_b.astype(np.float32), ohb=ohb, ky=ky, kx=kx)


def _core_inputs(core, fine, coarse, wd):
    """Build the in_map for one core."""
    b, half = core // 2, core % 2
    r0 = 32 * half
    fb = np.asarray(fine[b], np.float32).reshape(H * W, C)
    # halo rows r0-1 .. r0+32 with zero pad outside the image
    halo = np.zeros((34, W, C), np.float32)
    lo, hi = r0 - 1, r0 + 33
    src_lo, src_hi = max(lo, 0), min(hi, H)
    halo[src_lo - lo:src_hi - lo] = np.asarray(fine[b], np.float32)[src_lo:src_hi]
    # coarse rows u in [0,34): true row (r0//2 - 1 + u) mod 32; channels
    # permuted so partition pairing matches the packed table.
    cb = np.asarray(coarse[b], np.float32)[:, :, PERM256]      # [32,32,256]
    rows = (r0 // 2 - 1 + np.arange(34)) % 32
    coarse_rot = cb[rows].reshape(34 * 32, C)

    # y-pass per-partition weights [128, 8blk, 2(a,b)] for E and O slot sets
    we = np.zeros((128, 8, 2), np.float32)
    wo = np.zeros((128, 8, 2), np.float32)
    for blk in range(8):
        for i in range(4):
            se = blk * 4 + i                    # slot s = 2*se
            y = (r0 - 1 + 2 * se) % H           # odd fine row
            pa, pb = (1.0, 0.0) if y == H - 1 else (0.75, 0.25)
            we[32 * i:32 * (i + 1), blk, 0] = pa
            we[32 * i:32 * (i + 1), blk, 1] = pb
            so = blk * 4 + i                    # slot s = 2*so+1
            y = (r0 + 2 * so) % H               # even fine row
            pa, pb = (0.0, 1.0) if y == 0 else (0.25, 0.75)
            wo[32 * i:32 * (i + 1), blk, 0] = pa
            wo[32 * i:32 * (i + 1), blk, 1] = pb

    # YM: zero the om-halo row that lies outside the image: slot 0 (y=r0-1)
    # if r0==0 else slot 33 (y=r0+32==64)
    import ml_dtypes
    ym = np.ones((128, 34 * W), ml_dtypes.bfloat16)
    zslot = 0 if r0 == 0 else 33
    ym[:, zslot * W:(zslot + 1) * W] = 0.0

    pos = np.arange(POS, dtype=np.float32)
    yg = r0 + pos // W
    xg = pos % W
    # CY[r=(g,k), pos] = y_global(pos) + ky[k]; CX likewise
    kyr = np.tile(wd["ky"], DG)                                # [72] (g,k)
    kxr = np.tile(wd["kx"], DG)
    cy = (yg[None, :] + kyr[:, None]).astype(np.float32)
    cx = (xg[None, :] + kxr[:, None]).astype(np.float32)
    cr = np.full((72, 1), float(r0 - 1), np.float32)

    return {
        "fine_full": fb, "fine_halo": halo.reshape(HALO, C),
        "coarse_rot": coarse_rot,
        "w_att": wd["w_att"], "w_sel": wd["w_sel"],
        "wpb": wd["wpb"],
        "dcn_wb": wd["dcn_wb"], "dcn_b": wd["dcn_b"].reshape(F, 1),
        "ohb": wd["ohb"],
        "cy": cy, "cx": cx, "cr": cr, "ym": ym,
        "we": we.reshape(128, 16), "wo": wo.reshape(128, 16),
        "idt": np.eye(128, dtype=np.float32),
        "ones": np.ones((128, 1), np.float32),
    }


# ---------------------------------------------------------------- device
def _build_nc():
    nc = bacc.Bacc("TRN2", target_bir_lowering=False, debug=False)
    dt = nc.dram_tensor
    fine_full = dt("fine_full", [H * W, C], FP32, kind="ExternalInput").ap()
    fine_halo = dt("fine_halo", [HALO, C], FP32, kind="ExternalInput").ap()
    coarse_rot = dt("coarse_rot", [34 * 32, C], FP32, kind="ExternalInput").ap()
    w_att = dt("w_att", [C, C], FP32, kind="ExternalInput").ap()
    w_sel = dt("w_sel", [C, F], FP32, kind="ExternalInput").ap()
    wpb = dt("wpb", [KK, 2 * F, OMF], BF16, kind="ExternalInput").ap()
    dcn_wb = dt("dcn_wb", [KK, 2, 128, F], BF16, kind="ExternalInput").ap()
    dcn_b = dt("dcn_b", [F, 1], FP32, kind="ExternalInput").ap()
    ohb = dt("ohb", [KK, 72, 128], BF16, kind="ExternalInput").ap()
    cy_d = dt("cy", [72, POS], FP32, kind="ExternalInput").ap()
    cx_d = dt("cx", [72, POS], FP32, kind="ExternalInput").ap()
    cr_d = dt("cr", [72, 1], FP32, kind="ExternalInput").ap()
    ym_d = dt("ym", [128, HALO], BF16, kind="ExternalInput").ap()
    we_d = dt("we", [128, 16], FP32, kind="ExternalInput").ap()
    wo_d = dt("wo", [128, 16], FP32, kind="ExternalInput").ap()
    idt_d = dt("idt", [128, 128], FP32, kind="ExternalInput").ap()
    ones_d = dt("ones", [128, 1], FP32, kind="ExternalInput").ap()
    out_d = dt("out", [POS, C], FP32, kind="ExternalOutput").ap()
    idx_scr = dt("idx_scr", [8, 16, KK, 128], I16).ap()

    with TileCtx(nc) as tc, ExitStack() as ctx:
        v, s, pe, gp = nc.vector, nc.scalar, nc.tensor, nc.gpsimd
        pool = lambda name, bufs: ctx.enter_context(tc.tile_pool(name=name, bufs=bufs))

        cst = pool("cst", 1)
        idt = cst.tile([128, 128], FP32); nc.sync.dma_start(idt[:], idt_d)
        ones = cst.tile([128, 1], FP32); nc.sync.dma_start(ones[:], ones_d)
        crt = cst.tile([72, 1], FP32); nc.sync.dma_start(crt[:], cr_d)
        dcnbt = cst.tile([128, 2], FP32)
        nc.sync.dma_start(dcnbt[:], dcn_b.rearrange("(c p) one -> p (c one)", c=2))
        oht = cst.tile([72, KK * 128], BF16)
        nc.sync.dma_start(
            oht[:].rearrange("r (k p) -> r k p", k=KK),
            ohb.rearrange("k r p -> r k p"))

        # persistent: CT patch table, fcal, AL, weights+indices
        big = pool("big", 1)
        CT = big.tile([128, H * W * 8], BF16, name="ct", tag="ct")
        fcal = [big.tile([128, HALO], BF16, name=f"fcal{i}", tag=f"fcal{i}")
                for i in range(2)]
        AL = [big.tile([128, POS], FP32, name=f"al{i}", tag=f"al{i}")
              for i in range(2)]
        W4i = big.tile([72, 4 * POS], BF16)
        IW = big.tile([128, KK * 128], I16, name="iw")

        # ---- scope A: GAP + attention + fine transpose + fine_cal
        with tc.tile_pool(name="stage1", bufs=1) as st1, \
             tc.tile_pool(name="ld", bufs=3) as ld, \
             tc.tile_pool(name="tp_ps", bufs=2, space="PSUM") as tp_ps:
            fht = [st1.tile([128, HALO], BF16, name=f"fht{i}",
                            tag=f"fht{i}") for i in range(2)]
            wselb = st1.tile([128, 2 * F], BF16)
            gap = st1.tile([128, 2], FP32)
            sct = st1.tile([128, 2], FP32)

            # ---- GAP via ones-matmul over fine_full (batched loads)
            with tc.tile_pool(name="gap_ps", bufs=1, space="PSUM") as gpp:
                gap_ps = [gpp.tile([128, 1], FP32, name=f"gps{i}")
                          for i in range(2)]
                for i in range(8):
                    t = ld.tile([128, 4 * C], FP32, tag="fln")
                    nc.sync.dma_start(
                        t[:].rearrange("p (i c) -> p i c", i=4),
                        fine_full[512 * i:512 * (i + 1)].rearrange(
                            "(i p) c -> p i c", p=128))
                    tv = t[:].rearrange("p (i c) -> p i c", i=4)
                    for j in range(4):
                        for h_ in range(2):
                            pe.matmul(gap_ps[h_][:],
                                      tv[:, j, 128 * h_:128 * (h_ + 1)],
                                      ones[:],
                                      start=(i == 0 and j == 0),
                                      stop=(i == 7 and j == 3))
                for h_ in range(2):
                    s.copy(gap[:, h_:h_ + 1], gap_ps[h_][:])

            # ---- attn -> wselb
            with tc.tile_pool(name="att", bufs=2) as attp, \
                 tc.tile_pool(name="att_ps", bufs=1, space="PSUM") as atpp:
                for fc in range(2):
                    aps = atpp.tile([128, 1], FP32, tag="aps")
                    for cc in range(2):
                        wt = attp.tile([128, 128], FP32, tag="watt")
                        nc.sync.dma_start(
                            wt[:], w_att[128 * cc:128 * (cc + 1),
                                         128 * fc:128 * (fc + 1)])
                        pe.matmul(aps[:], wt[:], gap[:, cc:cc + 1],
                                  start=(cc == 0), stop=(cc == 1))
                    s.activation(sct[:, fc:fc + 1], aps[:], AF.Sigmoid)
                v.tensor_scalar(sct[:], sct[:], 1.0, None, OP.add)
                for cc in range(2):
                    wt = attp.tile([128, F], FP32, tag="wselld")
                    nc.sync.dma_start(wt[:],
                                      w_sel[128 * cc:128 * (cc + 1), :])
                    v.tensor_scalar(wselb[:, F * cc:F * (cc + 1)], wt[:],
                                    sct[:, cc:cc + 1], None, OP.mult)

            # ---- transpose fine_halo -> fht (bf16), batched loads
            for blk in range(3):
                nb = 6 if blk < 2 else 5
                r0_ = 6 * blk
                t = ld.tile([128, 6 * C], FP32, tag="fhl")
                nc.sync.dma_start(
                    t[:].rearrange("p (i c) -> p i c", i=6)[:, :nb, :],
                    fine_halo[128 * r0_:128 * (r0_ + nb)].rearrange(
                        "(i p) c -> p i c", p=128))
                tv = t[:].rearrange("p (i c) -> p i c", i=6)
                for j in range(nb):
                    i = r0_ + j
                    for cc in range(2):
                        ps = tp_ps.tile([128, 128], FP32, tag="tp")
                        pe.transpose(ps[:], tv[:, j, 128 * cc:128 * (cc + 1)],
                                     idt[:])
                        s.copy(fht[cc][:, 128 * i:128 * (i + 1)], ps[:])

            # ---- fine_cal = wselb.T @ fht
            with tc.tile_pool(name="fc_ps", bufs=2, space="PSUM") as fc_ps:
                for fc_ in range(2):
                    for pblk in range(5):
                        n0 = 512 * pblk
                        n1 = min(n0 + 512, HALO)
                        ps = fc_ps.tile([128, 512], FP32, tag="fc")
                        for cc in range(2):
                            pe.matmul(ps[:, :n1 - n0],
                                      wselb[:, F * cc + 128 * fc_:
                                            F * cc + 128 * fc_ + 128],
                                      fht[cc][:, n0:n1],
                                      start=(cc == 0), stop=(cc == 1))
                        s.copy(fcal[fc_][:, n0:n1], ps[:, :n1 - n0])

        # ---- scope B: coarse y-pass, transpose, x-pass -> CT patch table
        # CT layout: [p, q=(s*64+x), e=8]:
        #   e=4*cc+0: v(s, x)    e=4*cc+1: v(s, x+1)
        #   e=4*cc+2: v(s+1, x)  e=4*cc+3: v(s+1, x+1)
        # for channel half cc (cc=0: PERM[p], cc=1: PERM[p]+16);
        # s+1 wraps to slot 0 (rotation is cyclic in global rows).
        with tc.tile_pool(name="yeyo", bufs=1) as yeyo, \
             tc.tile_pool(name="tp_ps2", bufs=2, space="PSUM") as tp_ps2:
            wet = yeyo.tile([128, 16], FP32); nc.sync.dma_start(wet[:], we_d)
            wot = yeyo.tile([128, 16], FP32); nc.sync.dma_start(wot[:], wo_d)
            ye = yeyo.tile([128, 8 * C], FP32)
            yo = yeyo.tile([128, 8 * C], FP32)
            yev = ye[:].rearrange("p (i c) -> p i c", i=8)
            yov = yo[:].rearrange("p (i c) -> p i c", i=8)
            with tc.tile_pool(name="crot", bufs=1) as crotp:
                crA = crotp.tile([128, 8 * C], FP32)
                crB = crotp.tile([128, 9 * C], FP32)
                crAv = crA[:].rearrange("p (i c) -> p i c", i=8)
                crBv = crB[:].rearrange("p (i c) -> p i c", i=9)
                nc.sync.dma_start(
                    crAv,
                    coarse_rot[:1024].rearrange("(i p) c -> p i c", p=128))
                nc.sync.dma_start(
                    crBv[:, 0:8, :],
                    coarse_rot[32:1056].rearrange("(i p) c -> p i c", p=128))
                nc.sync.dma_start(crBv[0:32, 8, :], coarse_rot[1056:1088])
                for blk in range(8):
                    for (dstv, wtile) in ((yev, wet), (yov, wot)):
                        tb = crotp.tile([128, C], FP32, tag="yt")
                        v.tensor_scalar(
                            tb[:], crBv[:, blk, :],
                            wtile[:, 2 * blk + 1:2 * blk + 2],
                            None, OP.mult)
                        v.scalar_tensor_tensor(
                            dstv[:, blk, :], crAv[:, blk, :],
                            wtile[:, 2 * blk:2 * blk + 1], tb[:],
                            OP.mult, OP.add)
            with tc.tile_pool(name="cupf", bufs=1) as cupf:
                cmid = [cupf.tile([128, H * 32], BF16, name=f"cmid{i}",
                                  tag=f"cmid{i}") for i in range(2)]
                for ysrc, par in ((yev, 0), (yov, 1)):
                    for i in range(8):
                        for cc in range(2):
                            ps = tp_ps2.tile([128, 128], FP32, tag="tp")
                            pe.transpose(
                                ps[:],
                                ysrc[:, i, 128 * cc:128 * (cc + 1)],
                                idt[:])
                            dst = cmid[cc][:].rearrange(
                                "p (s2 two x) -> p s2 two x",
                                two=2, x=32)
                            s.copy(dst[:, 4 * i:4 * (i + 1), par, :],
                                   ps[:])
                ctq = CT[:].rearrange("p (q e) -> p q e", e=8)
                ct3 = CT[:].rearrange("p (s x e) -> p s x e", s=H, e=8)
                for cc in range(2):
                    mid = cmid[cc][:].rearrange("p (s x) -> p s x", s=H)
                    out3 = ct3[:, :, :, 4 * cc]
                    a = cupf.tile([128, H * 32], BF16, tag="xq")
                    v.tensor_scalar(a[:], cmid[cc][:], 0.25, None, OP.mult)
                    a3 = a[:].rearrange("p (s x) -> p s x", s=H)
                    v.scalar_tensor_tensor(out3[:, :, 2:64:2],
                                           mid[:, :, 1:], 0.75,
                                           a3[:, :, 0:31],
                                           OP.mult, OP.add)
                    v.tensor_copy(out3[:, :, 0:1], mid[:, :, 0:1])
                    v.scalar_tensor_tensor(out3[:, :, 1:63:2],
                                           mid[:, :, 0:31], 0.75,
                                           a3[:, :, 1:32],
                                           OP.mult, OP.add)
                    v.tensor_copy(out3[:, :, 63:64], mid[:, :, 31:32])
                    # x-pair slot: e+1 at q = e+0 at q+1
                    v.tensor_copy(ctq[:, 0:H * W - 1, 4 * cc + 1],
                                  ctq[:, 1:H * W, 4 * cc])
                    v.tensor_copy(ctq[:, H * W - 1:H * W, 4 * cc + 1],
                                  ctq[:, H * W - 1:H * W, 4 * cc])
                    # next-row slots: e+2:e+4 at (s,x) = e:e+2 at (s+1,x);
                    # s=63 wraps to slot 0
                    v.tensor_copy(ct3[:, 0:H - 1, :, 4 * cc + 2:4 * cc + 4],
                                  ct3[:, 1:H, :, 4 * cc:4 * cc + 2])
                    v.tensor_copy(ct3[:, H - 1:H, :, 4 * cc + 2:4 * cc + 4],
                                  ct3[:, 0:1, :, 4 * cc:4 * cc + 2])

        # ---- scope C: fused om conv + index/weight math (per 512-pos block)
        with tc.tile_pool(name="ppad", bufs=1) as ppad, \
             tc.tile_pool(name="om_ps", bufs=2, space="PSUM") as om_ps, \
             tc.tile_pool(name="mth", bufs=1) as mth, \
             tc.tile_pool(name="cycx", bufs=2) as cycx:
            ymt = ppad.tile([128, HALO], BF16)
            nc.sync.dma_start(ymt[:], ym_d)
            ptf = [ppad.tile([128, 34 * 66], BF16, name=f"ptf{i}",
                             tag=f"ptf{i}") for i in range(2)]
            ptc = [ppad.tile([128, 34 * 66], BF16, name=f"ptc{i}",
                             tag=f"ptc{i}") for i in range(2)]
            for i in range(2):
                gp.memset(ptf[i][:], 0.0)
                v.tensor_tensor(
                    ptf[i][:].rearrange("p (y x) -> p y x", y=34)[:, :, 1:65],
                    fcal[i][:].rearrange("p (y x) -> p y x", y=34),
                    ymt[:].rearrange("p (y x) -> p y x", y=34),
                    OP.mult)
                gp.memset(ptc[i][:], 0.0)
                ct3 = CT[:].rearrange("p (s x e) -> p s x e", s=H, e=8)
                v.tensor_tensor(
                    ptc[i][:].rearrange("p (y x) -> p y x", y=34)[:, :, 1:65],
                    ct3[:, 0:34, :, 4 * i],
                    ymt[:].rearrange("p (y x) -> p y x", y=34),
                    OP.mult)
            womt = ppad.tile([128, KK * 4 * OMF], BF16)
            nc.sync.dma_start(
                womt[:].rearrange("p (k c f) -> p k c f", k=KK, c=4),
                wpb.rearrange("k (c p) f -> p k c f", p=128))
            womv = womt[:].rearrange("p (k c f) -> p k c f", k=KK, c=4)

            CH = 512
            tn = lambda nm: mth.tile([72, CH], FP32, name=nm, tag=nm)
            ti = mth.tile([72, CH], I32)
            tf, tg, XP, FL, FR, X1 = (tn(x) for x in
                                      ("tf", "tg", "XP", "FL", "FR", "X1"))
            VA, VB, WY0, WY1, WR0, WR1, SSc, A0, A1, WX0, WX1 = (
                tn(x) for x in ("VA", "VB", "WY0", "WY1", "WR0", "WR1",
                                "SSc", "A0", "A1", "WX0", "WX1"))
            MSc = mth.tile([72, CH], FP32, name="MSc", tag="MSc")
            IUS = mth.tile([72, POS], I16, name="ius", tag="ius")

            def floor_(dst, srcp):
                v.tensor_copy(ti[:], srcp[:])
                v.tensor_copy(tf[:], ti[:])
                v.tensor_tensor(tg[:], tf[:], srcp[:], OP.is_gt)
                v.tensor_tensor(dst[:], tf[:], tg[:], OP.subtract)

            W4v = W4i[:].rearrange("p (n e) -> p n e", e=4)
            for pblk in range(4):
                # om conv for this 512-position block -> PSUM
                psD = om_ps.tile([72, 512], FP32, tag="omD")
                psX = om_ps.tile([72, 512], FP32, tag="omX")
                psM = om_ps.tile([72, 512], FP32, tag="omM")
                for k in range(KK):
                    dy, dx = k // 3 - 1, k % 3 - 1
                    for cc in range(4):
                        pt_ = (ptf[cc] if cc < 2 else ptc[cc - 2])
                        rv = pt_[:].rearrange("p (y x) -> p y x", y=34)[
                            :, 1 + 8 * pblk + dy:9 + 8 * pblk + dy,
                            1 + dx:65 + dx]
                        wv = womv[:, k, cc, :]
                        first = (k == 0 and cc == 0)
                        last = (k == KK - 1 and cc == 3)
                        pe.matmul(psD[:], wv[:, 0:72], rv,
                                  start=first, stop=last)
                        pe.matmul(psX[:], wv[:, 72:144], rv,
                                  start=first, stop=last)
                        pe.matmul(psM[:], wv[:, 144:216], rv,
                                  start=first, stop=last)
                s.activation(MSc[:], psM[:], AF.Sigmoid)

                # index & weight math for this block (reads psD/psX direct)
                ch = pblk
                c0 = CH * ch
                sl = slice(c0, c0 + CH)
                cyt = cycx.tile([72, CH], FP32, tag="cyt")
                nc.sync.dma_start(cyt[:], cy_d[:, sl])
                cxt = cycx.tile([72, CH], FP32, tag="cxt")
                nc.sync.dma_start(cxt[:], cx_d[:, sl])
                # y side
                v.tensor_tensor(XP[:], psD[:], cyt[:], OP.add)
                floor_(FL, XP)
                v.tensor_tensor(FR[:], XP[:], FL[:], OP.subtract)
                v.tensor_scalar(X1[:], FL[:], 1.0, None, OP.add)
                v.tensor_scalar(VA[:], FL[:], 0.0, None, OP.is_ge)
                v.tensor_scalar(tg[:], FL[:], 63.0, None, OP.is_le)
                v.tensor_tensor(VA[:], VA[:], tg[:], OP.mult)
                v.tensor_scalar(VB[:], X1[:], 0.0, None, OP.is_ge)
                v.tensor_scalar(tg[:], X1[:], 63.0, None, OP.is_le)
                v.tensor_tensor(VB[:], VB[:], tg[:], OP.mult)
                v.tensor_scalar(tf[:], FR[:], -1.0, 1.0, OP.mult, OP.add)
                v.tensor_tensor(tf[:], tf[:], VA[:], OP.mult)
                v.tensor_tensor(WY0[:], tf[:], MSc[:], OP.mult)
                v.tensor_tensor(tf[:], FR[:], VB[:], OP.mult)
                v.tensor_tensor(WY1[:], tf[:], MSc[:], OP.mult)
                # SY = clip(FL, 0, 62); row weights via equality masks
                v.tensor_scalar(SSc[:], FL[:], 0.0, 62.0, OP.max, OP.min)
                v.tensor_tensor(tf[:], SSc[:], FL[:], OP.is_equal)
                v.tensor_tensor(tg[:], SSc[:], X1[:], OP.is_equal)
                v.tensor_tensor(tf[:], tf[:], WY0[:], OP.mult)
                v.tensor_tensor(tg[:], tg[:], WY1[:], OP.mult)
                v.tensor_tensor(WR0[:], tf[:], tg[:], OP.add)
                v.tensor_scalar(VA[:], SSc[:], 1.0, None, OP.add)
                v.tensor_tensor(tf[:], VA[:], FL[:], OP.is_equal)
                v.tensor_tensor(tg[:], VA[:], X1[:], OP.is_equal)
                v.tensor_tensor(tf[:], tf[:], WY0[:], OP.mult)
                v.tensor_tensor(tg[:], tg[:], WY1[:], OP.mult)
                v.tensor_tensor(WR1[:], tf[:], tg[:], OP.add)
                # rotated row slot: rot = SY - crt (+64 if negative)
                v.tensor_scalar(tf[:], SSc[:], crt[:, 0:1], None, OP.subtract)
                v.tensor_scalar(tg[:], tf[:], 0.0, None, OP.is_lt)
                v.scalar_tensor_tensor(tf[:], tg[:], 64.0, tf[:],
                                       OP.mult, OP.add)
                v.tensor_scalar(WY0[:], tf[:], 64.0, None, OP.mult)
                # x side
                v.tensor_tensor(XP[:], psX[:], cxt[:], OP.add)
                floor_(FL, XP)
                v.tensor_tensor(FR[:], XP[:], FL[:], OP.subtract)
                v.tensor_scalar(X1[:], FL[:], 1.0, None, OP.add)
                v.tensor_scalar(VA[:], FL[:], 0.0, None, OP.is_ge)
                v.tensor_scalar(tg[:], FL[:], 63.0, None, OP.is_le)
                v.tensor_tensor(VA[:], VA[:], tg[:], OP.mult)
                v.tensor_scalar(VB[:], X1[:], 0.0, None, OP.is_ge)
                v.tensor_scalar(tg[:], X1[:], 63.0, None, OP.is_le)
                v.tensor_tensor(VB[:], VB[:], tg[:], OP.mult)
                v.tensor_scalar(tf[:], FR[:], -1.0, 1.0, OP.mult, OP.add)
                v.tensor_tensor(A0[:], tf[:], VA[:], OP.mult)
                v.tensor_tensor(A1[:], FR[:], VB[:], OP.mult)
                v.tensor_scalar(SSc[:], FL[:], 0.0, 62.0, OP.max, OP.min)
                v.tensor_tensor(tf[:], SSc[:], FL[:], OP.is_equal)
                v.tensor_tensor(tg[:], SSc[:], X1[:], OP.is_equal)
                v.tensor_tensor(tf[:], tf[:], A0[:], OP.mult)
                v.tensor_tensor(tg[:], tg[:], A1[:], OP.mult)
                v.tensor_tensor(WX0[:], tf[:], tg[:], OP.add)
                v.tensor_scalar(VA[:], SSc[:], 1.0, None, OP.add)
                v.tensor_tensor(tf[:], VA[:], FL[:], OP.is_equal)
                v.tensor_tensor(tg[:], VA[:], X1[:], OP.is_equal)
                v.tensor_tensor(tf[:], tf[:], A0[:], OP.mult)
                v.tensor_tensor(tg[:], tg[:], A1[:], OP.mult)
                v.tensor_tensor(WX1[:], tf[:], tg[:], OP.add)
                # q = rot*64 + SX -> IUS (wrapped free order)
                v.tensor_tensor(tf[:], WY0[:], SSc[:], OP.add)
                ncol = CH // 16
                dstv = IUS[:].rearrange(
                    "p (s c) -> p s c", s=16)[
                    :, :, ncol * ch:ncol * (ch + 1)].rearrange(
                    "p s c -> p c s")
                v.tensor_copy(dstv,
                              tf[:].rearrange("p (c s) -> p c s", s=16))
                # corner weights -> W4i
                v.tensor_tensor(W4v[:, sl, 0], WR0[:], WX0[:], OP.mult)
                v.tensor_tensor(W4v[:, sl, 1], WR0[:], WX1[:], OP.mult)
                v.tensor_tensor(W4v[:, sl, 2], WR1[:], WX0[:], OP.mult)
                v.tensor_tensor(W4v[:, sl, 3], WR1[:], WX1[:], OP.mult)

            # ---- idx bounce: per-group wrapped layout
            for g_ in range(8):
                nc.sync.dma_start(
                    idx_scr[g_].rearrange("p k col -> k p col"),
                    IUS[9 * g_:9 * (g_ + 1), :]
                    .rearrange("k (p col) -> k p col", p=16))
            nc.sync.dma_start(
                IW[:],
                idx_scr.rearrange("g p k col -> (g p) (k col)"))

        # ---- main loop
        with tc.tile_pool(name="dcnw", bufs=1) as dcnw_p, \
             tc.tile_pool(name="gat", bufs=3) as gat, \
             tc.tile_pool(name="wrep", bufs=2) as wrep, \
             tc.tile_pool(name="rep_ps", bufs=2, space="PSUM") as rep_ps, \
             tc.tile_pool(name="dcn_ps", bufs=1, space="PSUM") as dcn_ps, \
             tc.tile_pool(name="prod", bufs=1) as prod_p, \
             tc.tile_pool(name="val", bufs=2) as val_p:
            dwt = dcnw_p.tile([128, KK * 2 * F], BF16)
            nc.sync.dma_start(
                dwt[:].rearrange("p (k h f) -> p k h f", k=KK, h=2),
                dcn_wb.rearrange("k h p f -> p k h f"))
            dwv = dwt[:].rearrange("p (k h f) -> p k h f", k=KK, h=2)
            ctv = CT[:].rearrange("p (q e) -> p q e", e=8)

            def emit_rep(ph, k):
                """replicated 4-corner weights [128, PPOS, 4] for (ph, k)."""
                wre = wrep.tile([128, 4 * PPOS], BF16, tag="wr")
                for ns in range(8):
                    ps = rep_ps.tile([128, 512], FP32, tag="rep")
                    pe.matmul(ps[:], oht[:, 128 * k:128 * (k + 1)],
                              W4i[:, 4096 * ph + 512 * ns:
                                  4096 * ph + 512 * (ns + 1)],
                              start=True, stop=True)
                    s.copy(wre[:, 512 * ns:512 * (ns + 1)], ps[:])
                return wre

            wre_next = emit_rep(0, 0)
            for ph in range(NPH):
                p0 = PPOS * ph
                dps = [dcn_ps.tile([128, 512], FP32, name=f"dcn{ph}_{i}",
                                   tag=f"dcn{i}") for i in range(4)]
                for k in range(KK):
                    wre = wre_next
                    g_ = gat.tile([128, PPOS, 8], BF16, tag="g")
                    gp.ap_gather(
                        g_[:], ctv,
                        IW[:, 128 * k + 64 * ph:128 * k + 64 * ph + 64],
                        channels=128, num_elems=H * W, d=8,
                        num_idxs=PPOS)
                    # pre-emit next iteration's replication so PE keeps
                    # ahead of the serial gather stream
                    if not (ph == NPH - 1 and k == KK - 1):
                        nk = (k + 1) % KK
                        nph = ph + 1 if nk == 0 else ph
                        wre_next = emit_rep(nph, nk)
                    PA = prod_p.tile([128, PPOS, 4], BF16, tag="pa")
                    PB = prod_p.tile([128, PPOS, 4], BF16, tag="pb")
                    valA = val_p.tile([128, PPOS], BF16, tag="valA")
                    valB = val_p.tile([128, PPOS], BF16, tag="valB")
                    wr4 = wre[:].rearrange("p (n e) -> p n e", e=4)
                    v.tensor_tensor(PA[:], g_[:, :, 0:4], wr4, OP.mult)
                    v.tensor_tensor(PB[:], g_[:, :, 4:8], wr4, OP.mult)
                    v.tensor_tensor(PA[:, :, 0:2], PA[:, :, 0:2],
                                    PA[:, :, 2:4], OP.add)
                    v.tensor_tensor(valA[:], PA[:, :, 0], PA[:, :, 1],
                                    OP.add)
                    v.tensor_tensor(PB[:, :, 0:2], PB[:, :, 0:2],
                                    PB[:, :, 2:4], OP.add)
                    v.tensor_tensor(valB[:], PB[:, :, 0], PB[:, :, 1],
                                    OP.add)
                    first = (k == 0)
                    last = (k == KK - 1)
                    for h_, vv in ((0, valA), (1, valB)):
                        for fc_ in range(2):
                            for ns in range(2):
                                pe.matmul(
                                    dps[2 * fc_ + ns][:],
                                    dwv[:, k, h_,
                                        128 * fc_:128 * (fc_ + 1)],
                                    vv[:, 512 * ns:512 * (ns + 1)],
                                    start=(first and h_ == 0),
                                    stop=(last and h_ == 1))
                for fc_ in range(2):
                    for ns in range(2):
                        s.activation(
                            AL[fc_][:, p0 + 512 * ns:p0 + 512 * (ns + 1)],
                            dps[2 * fc_ + ns][:], AF.Relu,
                            bias=dcnbt[:, fc_:fc_ + 1])

        # ---- final add + transpose + out
        with tc.tile_pool(name="fin_ps", bufs=2, space="PSUM") as fin_ps, \
             tc.tile_pool(name="fin_sb", bufs=2) as fin_sb:
            for fc_ in range(2):
                v.tensor_tensor(AL[fc_][:], AL[fc_][:],
                                fcal[fc_][:, W:W + POS], OP.add)
            for i in range(16):
                ot = fin_sb.tile([128, C], FP32, tag="ot")
                for fc_ in range(2):
                    ps = fin_ps.tile([128, 128], FP32, tag="fin")
                    pe.transpose(ps[:], AL[fc_][:, 128 * i:128 * (i + 1)],
                                 idt[:])
                    s.copy(ot[:, 128 * fc_:128 * (fc_ + 1)], ps[:])
                nc.sync.dma_start(out_d[128 * i:128 * (i + 1), :], ot[:])

    nc.compile()
    return nc


TileCtx = tile.TileContext
_NC_CACHE = None


def _get_nc():
    global _NC_CACHE
    if _NC_CACHE is None:
        _NC_CACHE = _build_nc()
    return _NC_CACHE


def kernel(fine, coarse, attend_w, select_w, offset_w, om_w, om_b, dcn_w,
           dcn_b, _trace=False, _trace_kwargs=None):
    wd = _prep_weights(np.asarray(attend_w), np.asarray(select_w),
                       np.asarray(offset_w), np.asarray(om_w),
                       np.asarray(om_b), np.asarray(dcn_w), np.asarray(dcn_b))
    in_maps = [_core_inputs(c, np.asarray(fine), np.asarray(coarse), wd)
               for c in range(NCORES)]
    nc = _get_nc()
    kw = {}
    if _trace:
        import concourse.bass_utils as _bu
        _bu.upload_artifacts = lambda d: d
        kw = dict(trace=True, trace_kwargs=_trace_kwargs or {})
    res = run_bass_kernel_spmd(nc, in_maps, list(range(NCORES)), **kw)
    out = np.zeros((B, H, W, C), np.float32)
    for c in range(NCORES):
        b, half = c // 2, c % 2
        out[b, 32 * half:32 * half + 32] = res.results[c]["out"].reshape(32, W, C)
    if _trace:
        kernel._last_exec_ns = res.exec_time_ns
    return out


# revision 11
# speedup vs baseline: 2.4330x; 1.0046x over previous
"""DeformableFeatureAlignment fused Trainium2 kernel.

Sharding: data-parallel over (batch, row-half): core c handles batch c//2,
output rows 32*(c%2) .. +32. All weights replicated. The DCNv2 bilinear
gather runs on GPSIMD ap_gather against a channels-on-partitions
coarse_up patch table; row rotation makes the SPMD program core-independent.

Gather layout: one [128, 4096, 8] bf16 table where partition p=(g,j)
holds channels (32g+j, 32g+j+16) and each 8-wide block is the full 2x2
bilinear patch for both channels, so one index per (pos, tap, group)
fetches all 4 corners x 2 channels. Host permutes coarse channels (and
the matching om/dcn weight rows) so the pair lands on one partition.

Self-contained: hardcodes shapes from the problem spec.
"""
import sys

if "/opt/trn_rl_repo" not in sys.path:
    sys.path.insert(0, "/opt/trn_rl_repo")

import numpy as np
from contextlib import ExitStack

import concourse.bass as bass
import concourse.tile as tile
from concourse import bacc, mybir
from concourse.bass_utils import run_bass_kernel_spmd

FP32 = mybir.dt.float32
BF16 = mybir.dt.bfloat16
I32 = mybir.dt.int32
I16 = mybir.dt.int16
AF = mybir.ActivationFunctionType
OP = mybir.AluOpType

B, H, W, C, F = 4, 64, 64, 256, 256
DG, K, KK = 8, 3, 9
CPG = C // DG  # 32
NCORES = 8
POS = 32 * W          # 2048 positions per core (32 rows)
HALO = 34 * W         # 2176 halo positions (34 rows)
NPH = 2               # phases
PPOS = POS // NPH     # 1024 positions per phase
OMF = DG * 3 * KK     # 216

# channel permutation: partition p (0..127) holds channels
# (PERM[p], PERM[p]+16); both stay inside deformable group p//16.
PERM = np.array([32 * (p // 16) + p % 16 for p in range(128)])
PERM256 = np.concatenate([PERM, PERM + 16])


# ---------------------------------------------------------------- host prep
def _host_consts():
    """Core-independent constants."""
    om_perm = np.zeros(OMF, np.int64)
    for g in range(DG):
        for k in range(KK):
            om_perm[g * KK + k] = g * 2 * KK + 2 * k          # dy rows 0..71
            om_perm[72 + g * KK + k] = g * 2 * KK + 2 * k + 1  # dx rows 72..143
            om_perm[144 + g * KK + k] = 144 + g * KK + k       # mask rows
    ky = np.repeat(np.arange(K) - 1, K).astype(np.float32)     # k//3 - 1
    kx = np.tile(np.arange(K) - 1, K).astype(np.float32)       # k%3 - 1

    # one-hot replication lhsT: OH[k][row=(g,k'), p] = 1 iff k'==k and
    # g == p//16
    oh = np.zeros((KK, 72, 128), np.float32)
    for k in range(KK):
        for p in range(128):
            oh[k, (p // 16) * KK + k, p] = 1.0
    return om_perm, ky, kx, oh


def _prep_weights(attend_w, select_w, offset_w, om_w, om_b, dcn_w, dcn_b):
    om_perm, ky, kx, oh = _host_consts()
    w_att = (attend_w / (H * W)).astype(np.float32)            # fold GAP mean
    wp = np.einsum("co,dof->dcf", offset_w, om_w.reshape(KK, 2 * F, OMF))
    wp = wp.copy()
    wp[:, F:, :] *= 2.0                                        # coarse_up * 2
    wp[:, F:, :] = wp[:, F + PERM256, :]                       # permuted coarse
    wp = wp[:, :, om_perm].astype(np.float32)                  # [9,512,216]
    dcn_w9 = dcn_w.reshape(KK, C, F)
    # dwAB[k, h, p, f]: dcn weight rows in the packed-channel order
    dwAB = np.stack([dcn_w9[:, PERM, :], dcn_w9[:, PERM + 16, :]], 1)
    import ml_dtypes
    dcn_wb = dwAB.astype(ml_dtypes.bfloat16)                   # [KK,2,128,F]
    ohb = oh.astype(ml_dtypes.bfloat16)
    wpb = wp.astype(ml_dtypes.bfloat16)
    return dict(w_att=w_att, w_sel=select_w.astype(np.float32),
                wpb=wpb, dcn_wb=dcn_wb,
                dcn_b=dcn_b.astype(np.float32), ohb=ohb, ky=ky, kx=kx)


def _core_inputs(core, fine, coarse, wd):
    """Build the in_map for one core."""
    b, half = core // 2, core % 2
    r0 = 32 * half
    fb = np.asarray(fine[b], np.float32).reshape(H * W, C)
    # halo rows r0-1 .. r0+32 with zero pad outside the image
    halo = np.zeros((34, W, C), np.float32)
    lo, hi = r0 - 1, r0 + 33
    src_lo, src_hi = max(lo, 0), min(hi, H)
    halo[src_lo - lo:src_hi - lo] = np.asarray(fine[b], np.float32)[src_lo:src_hi]
    # coarse rows u in [0,34): true row (r0//2 - 1 + u) mod 32; channels
    # permuted so partition pairing matches the packed table.
    cb = np.asarray(coarse[b], np.float32)[:, :, PERM256]      # [32,32,256]
    rows = (r0 // 2 - 1 + np.arange(34)) % 32
    coarse_rot = cb[rows].reshape(34 * 32, C)

    # y-pass per-partition weights [128, 8blk, 2(a,b)] for E and O slot sets
    we = np.zeros((128, 8, 2), np.float32)
    wo = np.zeros((128, 8, 2), np.float32)
    for blk in range(8):
        for i in range(4):
            se = blk * 4 + i                    # slot s = 2*se
            y = (r0 - 1 + 2 * se) % H           # odd fine row
            pa, pb = (1.0, 0.0) if y == H - 1 else (0.75, 0.25)
            we[32 * i:32 * (i + 1), blk, 0] = pa
            we[32 * i:32 * (i + 1), blk, 1] = pb
            so = blk * 4 + i                    # slot s = 2*so+1
            y = (r0 + 2 * so) % H               # even fine row
            pa, pb = (0.0, 1.0) if y == 0 else (0.25, 0.75)
            wo[32 * i:32 * (i + 1), blk, 0] = pa
            wo[32 * i:32 * (i + 1), blk, 1] = pb

    # YM: zero the om-halo row that lies outside the image: slot 0 (y=r0-1)
    # if r0==0 else slot 33 (y=r0+32==64)
    import ml_dtypes
    ym = np.ones((128, 34 * W), ml_dtypes.bfloat16)
    zslot = 0 if r0 == 0 else 33
    ym[:, zslot * W:(zslot + 1) * W] = 0.0

    pos = np.arange(POS, dtype=np.float32)
    yg = r0 + pos // W
    xg = pos % W
    # CY[r=(g,k), pos] = y_global(pos) + ky[k]; CX likewise
    kyr = np.tile(wd["ky"], DG)                                # [72] (g,k)
    kxr = np.tile(wd["kx"], DG)
    cy = (yg[None, :] + kyr[:, None]).astype(np.float32)
    cx = (xg[None, :] + kxr[:, None]).astype(np.float32)
    cr = np.full((72, 1), float(r0 - 1), np.float32)

    return {
        "fine_full": fb, "fine_halo": halo.reshape(HALO, C),
        "coarse_rot": coarse_rot,
        "w_att": wd["w_att"], "w_sel": wd["w_sel"],
        "wpb": wd["wpb"],
        "dcn_wb": wd["dcn_wb"], "dcn_b": wd["dcn_b"].reshape(F, 1),
        "ohb": wd["ohb"],
        "cy": cy, "cx": cx, "cr": cr, "ym": ym,
        "we": we.reshape(128, 16), "wo": wo.reshape(128, 16),
        "idt": np.eye(128, dtype=np.float32),
        "ones": np.ones((128, 1), np.float32),
    }


# ---------------------------------------------------------------- device
def _build_nc():
    nc = bacc.Bacc("TRN2", target_bir_lowering=False, debug=False)
    dt = nc.dram_tensor
    fine_full = dt("fine_full", [H * W, C], FP32, kind="ExternalInput").ap()
    fine_halo = dt("fine_halo", [HALO, C], FP32, kind="ExternalInput").ap()
    coarse_rot = dt("coarse_rot", [34 * 32, C], FP32, kind="ExternalInput").ap()
    w_att = dt("w_att", [C, C], FP32, kind="ExternalInput").ap()
    w_sel = dt("w_sel", [C, F], FP32, kind="ExternalInput").ap()
    wpb = dt("wpb", [KK, 2 * F, OMF], BF16, kind="ExternalInput").ap()
    dcn_wb = dt("dcn_wb", [KK, 2, 128, F], BF16, kind="ExternalInput").ap()
    dcn_b = dt("dcn_b", [F, 1], FP32, kind="ExternalInput").ap()
    ohb = dt("ohb", [KK, 72, 128], BF16, kind="ExternalInput").ap()
    cy_d = dt("cy", [72, POS], FP32, kind="ExternalInput").ap()
    cx_d = dt("cx", [72, POS], FP32, kind="ExternalInput").ap()
    cr_d = dt("cr", [72, 1], FP32, kind="ExternalInput").ap()
    ym_d = dt("ym", [128, HALO], BF16, kind="ExternalInput").ap()
    we_d = dt("we", [128, 16], FP32, kind="ExternalInput").ap()
    wo_d = dt("wo", [128, 16], FP32, kind="ExternalInput").ap()
    idt_d = dt("idt", [128, 128], FP32, kind="ExternalInput").ap()
    ones_d = dt("ones", [128, 1], FP32, kind="ExternalInput").ap()
    out_d = dt("out", [POS, C], FP32, kind="ExternalOutput").ap()
    idx_scrA = dt("idx_scrA", [8, 16, KK, 64], I16).ap()
    idx_scrB = dt("idx_scrB", [8, 16, KK, 64], I16).ap()

    with TileCtx(nc) as tc, ExitStack() as ctx:
        v, s, pe, gp = nc.vector, nc.scalar, nc.tensor, nc.gpsimd
        pool = lambda name, bufs: ctx.enter_context(tc.tile_pool(name=name, bufs=bufs))

        cst = pool("cst", 1)
        idt = cst.tile([128, 128], FP32); nc.sync.dma_start(idt[:], idt_d)
        ones = cst.tile([128, 1], FP32); nc.sync.dma_start(ones[:], ones_d)
        crt = cst.tile([72, 1], FP32); nc.sync.dma_start(crt[:], cr_d)
        dcnbt = cst.tile([128, 2], FP32)
        nc.sync.dma_start(dcnbt[:], dcn_b.rearrange("(c p) one -> p (c one)", c=2))
        oht = cst.tile([72, KK * 128], BF16)
        nc.sync.dma_start(
            oht[:].rearrange("r (k p) -> r k p", k=KK),
            ohb.rearrange("k r p -> r k p"))

        # persistent: CT patch table, fcal, AL, weights+indices
        big = pool("big", 1)
        CT = big.tile([128, H * W * 8], BF16, name="ct", tag="ct")
        fcal = [big.tile([128, HALO], BF16, name=f"fcal{i}", tag=f"fcal{i}")
                for i in range(2)]
        AL = [big.tile([128, POS], FP32, name=f"al{i}", tag=f"al{i}")
              for i in range(2)]
        W4i = big.tile([72, 4 * POS], BF16)
        IWA = big.tile([128, KK * 64], I16, name="iwa")
        IWB = big.tile([128, KK * 64], I16, name="iwb")

        # ---- scope A: GAP + attention + fine transpose + fine_cal
        with tc.tile_pool(name="stage1", bufs=1) as st1, \
             tc.tile_pool(name="ld", bufs=3) as ld, \
             tc.tile_pool(name="tp_ps", bufs=2, space="PSUM") as tp_ps:
            fht = [st1.tile([128, HALO], BF16, name=f"fht{i}",
                            tag=f"fht{i}") for i in range(2)]
            wselb = st1.tile([128, 2 * F], BF16)
            gap = st1.tile([128, 2], FP32)
            sct = st1.tile([128, 2], FP32)

            # ---- GAP via ones-matmul over fine_full (batched loads)
            with tc.tile_pool(name="gap_ps", bufs=1, space="PSUM") as gpp:
                gap_ps = [gpp.tile([128, 1], FP32, name=f"gps{i}")
                          for i in range(2)]
                for i in range(8):
                    t = ld.tile([128, 4 * C], FP32, tag="fln")
                    nc.sync.dma_start(
                        t[:].rearrange("p (i c) -> p i c", i=4),
                        fine_full[512 * i:512 * (i + 1)].rearrange(
                            "(i p) c -> p i c", p=128))
                    tv = t[:].rearrange("p (i c) -> p i c", i=4)
                    for j in range(4):
                        for h_ in range(2):
                            pe.matmul(gap_ps[h_][:],
                                      tv[:, j, 128 * h_:128 * (h_ + 1)],
                                      ones[:],
                                      start=(i == 0 and j == 0),
                                      stop=(i == 7 and j == 3))
                for h_ in range(2):
                    s.copy(gap[:, h_:h_ + 1], gap_ps[h_][:])

            # ---- attn -> wselb
            with tc.tile_pool(name="att", bufs=2) as attp, \
                 tc.tile_pool(name="att_ps", bufs=1, space="PSUM") as atpp:
                for fc in range(2):
                    aps = atpp.tile([128, 1], FP32, tag="aps")
                    for cc in range(2):
                        wt = attp.tile([128, 128], FP32, tag="watt")
                        nc.sync.dma_start(
                            wt[:], w_att[128 * cc:128 * (cc + 1),
                                         128 * fc:128 * (fc + 1)])
                        pe.matmul(aps[:], wt[:], gap[:, cc:cc + 1],
                                  start=(cc == 0), stop=(cc == 1))
                    s.activation(sct[:, fc:fc + 1], aps[:], AF.Sigmoid)
                v.tensor_scalar(sct[:], sct[:], 1.0, None, OP.add)
                for cc in range(2):
                    wt = attp.tile([128, F], FP32, tag="wselld")
                    nc.sync.dma_start(wt[:],
                                      w_sel[128 * cc:128 * (cc + 1), :])
                    v.tensor_scalar(wselb[:, F * cc:F * (cc + 1)], wt[:],
                                    sct[:, cc:cc + 1], None, OP.mult)

            # ---- transpose fine_halo -> fht (bf16), batched loads
            for blk in range(3):
                nb = 6 if blk < 2 else 5
                r0_ = 6 * blk
                t = ld.tile([128, 6 * C], FP32, tag="fhl")
                nc.sync.dma_start(
                    t[:].rearrange("p (i c) -> p i c", i=6)[:, :nb, :],
                    fine_halo[128 * r0_:128 * (r0_ + nb)].rearrange(
                        "(i p) c -> p i c", p=128))
                tv = t[:].rearrange("p (i c) -> p i c", i=6)
                for j in range(nb):
                    i = r0_ + j
                    for cc in range(2):
                        ps = tp_ps.tile([128, 128], FP32, tag="tp")
                        pe.transpose(ps[:], tv[:, j, 128 * cc:128 * (cc + 1)],
                                     idt[:])
                        s.copy(fht[cc][:, 128 * i:128 * (i + 1)], ps[:])

            # ---- fine_cal = wselb.T @ fht
            with tc.tile_pool(name="fc_ps", bufs=2, space="PSUM") as fc_ps:
                for fc_ in range(2):
                    for pblk in range(5):
                        n0 = 512 * pblk
                        n1 = min(n0 + 512, HALO)
                        ps = fc_ps.tile([128, 512], FP32, tag="fc")
                        for cc in range(2):
                            pe.matmul(ps[:, :n1 - n0],
                                      wselb[:, F * cc + 128 * fc_:
                                            F * cc + 128 * fc_ + 128],
                                      fht[cc][:, n0:n1],
                                      start=(cc == 0), stop=(cc == 1))
                        s.copy(fcal[fc_][:, n0:n1], ps[:, :n1 - n0])

        # ---- scope B: coarse y-pass, transpose, x-pass -> CT patch table
        # CT layout: [p, q=(s*64+x), e=8]:
        #   e=4*cc+0: v(s, x)    e=4*cc+1: v(s, x+1)
        #   e=4*cc+2: v(s+1, x)  e=4*cc+3: v(s+1, x+1)
        # for channel half cc (cc=0: PERM[p], cc=1: PERM[p]+16);
        # s+1 wraps to slot 0 (rotation is cyclic in global rows).
        with tc.tile_pool(name="yeyo", bufs=1) as yeyo, \
             tc.tile_pool(name="tp_ps2", bufs=2, space="PSUM") as tp_ps2:
            wet = yeyo.tile([128, 16], FP32); nc.sync.dma_start(wet[:], we_d)
            wot = yeyo.tile([128, 16], FP32); nc.sync.dma_start(wot[:], wo_d)
            ye = yeyo.tile([128, 8 * C], FP32)
            yo = yeyo.tile([128, 8 * C], FP32)
            yev = ye[:].rearrange("p (i c) -> p i c", i=8)
            yov = yo[:].rearrange("p (i c) -> p i c", i=8)
            with tc.tile_pool(name="crot", bufs=1) as crotp:
                crA = crotp.tile([128, 8 * C], FP32)
                crB = crotp.tile([128, 9 * C], FP32)
                crAv = crA[:].rearrange("p (i c) -> p i c", i=8)
                crBv = crB[:].rearrange("p (i c) -> p i c", i=9)
                nc.sync.dma_start(
                    crAv,
                    coarse_rot[:1024].rearrange("(i p) c -> p i c", p=128))
                nc.sync.dma_start(
                    crBv[:, 0:8, :],
                    coarse_rot[32:1056].rearrange("(i p) c -> p i c", p=128))
                nc.sync.dma_start(crBv[0:32, 8, :], coarse_rot[1056:1088])
                for blk in range(8):
                    for (dstv, wtile) in ((yev, wet), (yov, wot)):
                        tb = crotp.tile([128, C], FP32, tag="yt")
                        v.tensor_scalar(
                            tb[:], crBv[:, blk, :],
                            wtile[:, 2 * blk + 1:2 * blk + 2],
                            None, OP.mult)
                        v.scalar_tensor_tensor(
                            dstv[:, blk, :], crAv[:, blk, :],
                            wtile[:, 2 * blk:2 * blk + 1], tb[:],
                            OP.mult, OP.add)
            with tc.tile_pool(name="cupf", bufs=1) as cupf:
                cmid = [cupf.tile([128, H * 32], BF16, name=f"cmid{i}",
                                  tag=f"cmid{i}") for i in range(2)]
                for ysrc, par in ((yev, 0), (yov, 1)):
                    for i in range(8):
                        for cc in range(2):
                            ps = tp_ps2.tile([128, 128], FP32, tag="tp")
                            pe.transpose(
                                ps[:],
                                ysrc[:, i, 128 * cc:128 * (cc + 1)],
                                idt[:])
                            dst = cmid[cc][:].rearrange(
                                "p (s2 two x) -> p s2 two x",
                                two=2, x=32)
                            s.copy(dst[:, 4 * i:4 * (i + 1), par, :],
                                   ps[:])
                ctq = CT[:].rearrange("p (q e) -> p q e", e=8)
                ct3 = CT[:].rearrange("p (s x e) -> p s x e", s=H, e=8)
                for cc in range(2):
                    mid = cmid[cc][:].rearrange("p (s x) -> p s x", s=H)
                    out3 = ct3[:, :, :, 4 * cc]
                    a = cupf.tile([128, H * 32], BF16, tag="xq")
                    v.tensor_scalar(a[:], cmid[cc][:], 0.25, None, OP.mult)
                    a3 = a[:].rearrange("p (s x) -> p s x", s=H)
                    v.scalar_tensor_tensor(out3[:, :, 2:64:2],
                                           mid[:, :, 1:], 0.75,
                                           a3[:, :, 0:31],
                                           OP.mult, OP.add)
                    v.tensor_copy(out3[:, :, 0:1], mid[:, :, 0:1])
                    v.scalar_tensor_tensor(out3[:, :, 1:63:2],
                                           mid[:, :, 0:31], 0.75,
                                           a3[:, :, 1:32],
                                           OP.mult, OP.add)
                    v.tensor_copy(out3[:, :, 63:64], mid[:, :, 31:32])
                    # x-pair slot: e+1 at q = e+0 at q+1
                    v.tensor_copy(ctq[:, 0:H * W - 1, 4 * cc + 1],
                                  ctq[:, 1:H * W, 4 * cc])
                    v.tensor_copy(ctq[:, H * W - 1:H * W, 4 * cc + 1],
                                  ctq[:, H * W - 1:H * W, 4 * cc])
                    # next-row slots: e+2:e+4 at (s,x) = e:e+2 at (s+1,x);
                    # s=63 wraps to slot 0
                    v.tensor_copy(ct3[:, 0:H - 1, :, 4 * cc + 2:4 * cc + 4],
                                  ct3[:, 1:H, :, 4 * cc:4 * cc + 2])
                    v.tensor_copy(ct3[:, H - 1:H, :, 4 * cc + 2:4 * cc + 4],
                                  ct3[:, 0:1, :, 4 * cc:4 * cc + 2])

        # ---- scope C: fused om conv + index/weight math (per 512-pos block)
        with tc.tile_pool(name="ppad", bufs=1) as ppad, \
             tc.tile_pool(name="om_ps", bufs=2, space="PSUM") as om_ps, \
             tc.tile_pool(name="mth", bufs=1) as mth, \
             tc.tile_pool(name="cycx", bufs=2) as cycx:
            ymt = ppad.tile([128, HALO], BF16)
            nc.sync.dma_start(ymt[:], ym_d)
            ptf = [ppad.tile([128, 34 * 66], BF16, name=f"ptf{i}",
                             tag=f"ptf{i}") for i in range(2)]
            ptc = [ppad.tile([128, 34 * 66], BF16, name=f"ptc{i}",
                             tag=f"ptc{i}") for i in range(2)]
            for i in range(2):
                gp.memset(ptf[i][:], 0.0)
                v.tensor_tensor(
                    ptf[i][:].rearrange("p (y x) -> p y x", y=34)[:, :, 1:65],
                    fcal[i][:].rearrange("p (y x) -> p y x", y=34),
                    ymt[:].rearrange("p (y x) -> p y x", y=34),
                    OP.mult)
                gp.memset(ptc[i][:], 0.0)
                ct3 = CT[:].rearrange("p (s x e) -> p s x e", s=H, e=8)
                v.tensor_tensor(
                    ptc[i][:].rearrange("p (y x) -> p y x", y=34)[:, :, 1:65],
                    ct3[:, 0:34, :, 4 * i],
                    ymt[:].rearrange("p (y x) -> p y x", y=34),
                    OP.mult)
            womt = ppad.tile([128, KK * 4 * OMF], BF16)
            nc.sync.dma_start(
                womt[:].rearrange("p (k c f) -> p k c f", k=KK, c=4),
                wpb.rearrange("k (c p) f -> p k c f", p=128))
            womv = womt[:].rearrange("p (k c f) -> p k c f", k=KK, c=4)

            CH = 512
            tn = lambda nm: mth.tile([72, CH], FP32, name=nm, tag=nm)
            ti = mth.tile([72, CH], I32)
            tf, tg, XP, FL, FR, X1 = (tn(x) for x in
                                      ("tf", "tg", "XP", "FL", "FR", "X1"))
            VA, VB, WY0, WY1, WR0, WR1, SSc, A0, A1, WX0, WX1 = (
                tn(x) for x in ("VA", "VB", "WY0", "WY1", "WR0", "WR1",
                                "SSc", "A0", "A1", "WX0", "WX1"))
            MSc = mth.tile([72, CH], FP32, name="MSc", tag="MSc")
            IUS = mth.tile([72, POS], I16, name="ius", tag="ius")

            def floor_(dst, srcp):
                v.tensor_copy(ti[:], srcp[:])
                v.tensor_copy(tf[:], ti[:])
                v.tensor_tensor(tg[:], tf[:], srcp[:], OP.is_gt)
                v.tensor_tensor(dst[:], tf[:], tg[:], OP.subtract)

            W4v = W4i[:].rearrange("p (n e) -> p n e", e=4)
            for pblk in range(4):
                # om conv for this 512-position block -> PSUM
                psD = om_ps.tile([72, 512], FP32, tag="omD")
                psX = om_ps.tile([72, 512], FP32, tag="omX")
                psM = om_ps.tile([72, 512], FP32, tag="omM")
                for k in range(KK):
                    dy, dx = k // 3 - 1, k % 3 - 1
                    for cc in range(4):
                        pt_ = (ptf[cc] if cc < 2 else ptc[cc - 2])
                        rv = pt_[:].rearrange("p (y x) -> p y x", y=34)[
                            :, 1 + 8 * pblk + dy:9 + 8 * pblk + dy,
                            1 + dx:65 + dx]
                        wv = womv[:, k, cc, :]
                        first = (k == 0 and cc == 0)
                        last = (k == KK - 1 and cc == 3)
                        pe.matmul(psD[:], wv[:, 0:72], rv,
                                  start=first, stop=last)
                        pe.matmul(psX[:], wv[:, 72:144], rv,
                                  start=first, stop=last)
                        pe.matmul(psM[:], wv[:, 144:216], rv,
                                  start=first, stop=last)
                s.activation(MSc[:], psM[:], AF.Sigmoid)

                # index & weight math for this block (reads psD/psX direct)
                ch = pblk
                c0 = CH * ch
                sl = slice(c0, c0 + CH)
                cyt = cycx.tile([72, CH], FP32, tag="cyt")
                nc.sync.dma_start(cyt[:], cy_d[:, sl])
                cxt = cycx.tile([72, CH], FP32, tag="cxt")
                nc.sync.dma_start(cxt[:], cx_d[:, sl])
                # y side
                v.tensor_tensor(XP[:], psD[:], cyt[:], OP.add)
                floor_(FL, XP)
                v.tensor_tensor(FR[:], XP[:], FL[:], OP.subtract)
                v.tensor_scalar(X1[:], FL[:], 1.0, None, OP.add)
                v.tensor_scalar(VA[:], FL[:], 0.0, None, OP.is_ge)
                v.tensor_scalar(tg[:], FL[:], 63.0, None, OP.is_le)
                v.tensor_tensor(VA[:], VA[:], tg[:], OP.mult)
                v.tensor_scalar(VB[:], X1[:], 0.0, None, OP.is_ge)
                v.tensor_scalar(tg[:], X1[:], 63.0, None, OP.is_le)
                v.tensor_tensor(VB[:], VB[:], tg[:], OP.mult)
                v.tensor_scalar(tf[:], FR[:], -1.0, 1.0, OP.mult, OP.add)
                v.tensor_tensor(tf[:], tf[:], VA[:], OP.mult)
                v.tensor_tensor(WY0[:], tf[:], MSc[:], OP.mult)
                v.tensor_tensor(tf[:], FR[:], VB[:], OP.mult)
                v.tensor_tensor(WY1[:], tf[:], MSc[:], OP.mult)
                # SY = clip(FL, 0, 62); row weights via equality masks
                v.tensor_scalar(SSc[:], FL[:], 0.0, 62.0, OP.max, OP.min)
                v.tensor_tensor(tf[:], SSc[:], FL[:], OP.is_equal)
                v.tensor_tensor(tg[:], SSc[:], X1[:], OP.is_equal)
                v.tensor_tensor(tf[:], tf[:], WY0[:], OP.mult)
                v.tensor_tensor(tg[:], tg[:], WY1[:], OP.mult)
                v.tensor_tensor(WR0[:], tf[:], tg[:], OP.add)
                v.tensor_scalar(VA[:], SSc[:], 1.0, None, OP.add)
                v.tensor_tensor(tf[:], VA[:], FL[:], OP.is_equal)
                v.tensor_tensor(tg[:], VA[:], X1[:], OP.is_equal)
                v.tensor_tensor(tf[:], tf[:], WY0[:], OP.mult)
                v.tensor_tensor(tg[:], tg[:], WY1[:], OP.mult)
                v.tensor_tensor(WR1[:], tf[:], tg[:], OP.add)
                # rotated row slot: rot = SY - crt (+64 if negative)
                v.tensor_scalar(tf[:], SSc[:], crt[:, 0:1], None, OP.subtract)
                v.tensor_scalar(tg[:], tf[:], 0.0, None, OP.is_lt)
                v.scalar_tensor_tensor(tf[:], tg[:], 64.0, tf[:],
                                       OP.mult, OP.add)
                v.tensor_scalar(WY0[:], tf[:], 64.0, None, OP.mult)
                # x side
                v.tensor_tensor(XP[:], psX[:], cxt[:], OP.add)
                floor_(FL, XP)
                v.tensor_tensor(FR[:], XP[:], FL[:], OP.subtract)
                v.tensor_scalar(X1[:], FL[:], 1.0, None, OP.add)
                v.tensor_scalar(VA[:], FL[:], 0.0, None, OP.is_ge)
                v.tensor_scalar(tg[:], FL[:], 63.0, None, OP.is_le)
                v.tensor_tensor(VA[:], VA[:], tg[:], OP.mult)
                v.tensor_scalar(VB[:], X1[:], 0.0, None, OP.is_ge)
                v.tensor_scalar(tg[:], X1[:], 63.0, None, OP.is_le)
                v.tensor_tensor(VB[:], VB[:], tg[:], OP.mult)
                v.tensor_scalar(tf[:], FR[:], -1.0, 1.0, OP.mult, OP.add)
                v.tensor_tensor(A0[:], tf[:], VA[:], OP.mult)
                v.tensor_tensor(A1[:], FR[:], VB[:], OP.mult)
                v.tensor_scalar(SSc[:], FL[:], 0.0, 62.0, OP.max, OP.min)
                v.tensor_tensor(tf[:], SSc[:], FL[:], OP.is_equal)
                v.tensor_tensor(tg[:], SSc[:], X1[:], OP.is_equal)
                v.tensor_tensor(tf[:], tf[:], A0[:], OP.mult)
                v.tensor_tensor(tg[:], tg[:], A1[:], OP.mult)
                v.tensor_tensor(WX0[:], tf[:], tg[:], OP.add)
                v.tensor_scalar(VA[:], SSc[:], 1.0, None, OP.add)
                v.tensor_tensor(tf[:], VA[:], FL[:], OP.is_equal)
                v.tensor_tensor(tg[:], VA[:], X1[:], OP.is_equal)
                v.tensor_tensor(tf[:], tf[:], A0[:], OP.mult)
                v.tensor_tensor(tg[:], tg[:], A1[:], OP.mult)
                v.tensor_tensor(WX1[:], tf[:], tg[:], OP.add)
                # q = rot*64 + SX -> IUS (wrapped free order)
                v.tensor_tensor(tf[:], WY0[:], SSc[:], OP.add)
                ncol = CH // 16
                dstv = IUS[:].rearrange(
                    "p (s c) -> p s c", s=16)[
                    :, :, ncol * ch:ncol * (ch + 1)].rearrange(
                    "p s c -> p c s")
                v.tensor_copy(dstv,
                              tf[:].rearrange("p (c s) -> p c s", s=16))
                # corner weights -> W4i
                v.tensor_tensor(W4v[:, sl, 0], WR0[:], WX0[:], OP.mult)
                v.tensor_tensor(W4v[:, sl, 1], WR0[:], WX1[:], OP.mult)
                v.tensor_tensor(W4v[:, sl, 2], WR1[:], WX0[:], OP.mult)
                v.tensor_tensor(W4v[:, sl, 3], WR1[:], WX1[:], OP.mult)
                # bounce this phase's indices as soon as its chunks done
                if pblk == 1 or pblk == 3:
                    scr = idx_scrA if pblk == 1 else idx_scrB
                    iwt = IWA if pblk == 1 else IWB
                    cl = slice(0, 64) if pblk == 1 else slice(64, 128)
                    for g_ in range(8):
                        nc.sync.dma_start(
                            scr[g_].rearrange("p k col -> k p col"),
                            IUS[9 * g_:9 * (g_ + 1), :]
                            .rearrange("k (p col) -> k p col", p=16)[:, :, cl])
                    nc.sync.dma_start(
                        iwt[:],
                        scr.rearrange("g p k col -> (g p) (k col)"))

        # ---- main loop
        with tc.tile_pool(name="dcnw", bufs=1) as dcnw_p, \
             tc.tile_pool(name="gat", bufs=2) as gat, \
             tc.tile_pool(name="wrep", bufs=2) as wrep, \
             tc.tile_pool(name="rep_ps", bufs=2, space="PSUM") as rep_ps, \
             tc.tile_pool(name="dcn_ps", bufs=1, space="PSUM") as dcn_ps, \
             tc.tile_pool(name="prod", bufs=1) as prod_p, \
             tc.tile_pool(name="val", bufs=2) as val_p:
            dwt = dcnw_p.tile([128, KK * 2 * F], BF16)
            nc.sync.dma_start(
                dwt[:].rearrange("p (k h f) -> p k h f", k=KK, h=2),
                dcn_wb.rearrange("k h p f -> p k h f"))
            dwv = dwt[:].rearrange("p (k h f) -> p k h f", k=KK, h=2)
            ctv = CT[:].rearrange("p (q e) -> p q e", e=8)

            def emit_rep(ph, k):
                """replicated 4-corner weights [128, PPOS, 4] for (ph, k)."""
                wre = wrep.tile([128, 4 * PPOS], BF16, tag="wr")
                for ns in range(8):
                    ps = rep_ps.tile([128, 512], FP32, tag="rep")
                    pe.matmul(ps[:], oht[:, 128 * k:128 * (k + 1)],
                              W4i[:, 4096 * ph + 512 * ns:
                                  4096 * ph + 512 * (ns + 1)],
                              start=True, stop=True)
                    s.copy(wre[:, 512 * ns:512 * (ns + 1)], ps[:])
                return wre

            wre_next = emit_rep(0, 0)
            for ph in range(NPH):
                p0 = PPOS * ph
                dps = [dcn_ps.tile([128, 512], FP32, name=f"dcn{ph}_{i}",
                                   tag=f"dcn{i}") for i in range(4)]
                for k in range(KK):
                    wre = wre_next
                    g_ = gat.tile([128, PPOS, 8], BF16, tag="g")
                    iwt = IWA if ph == 0 else IWB
                    gp.ap_gather(
                        g_[:], ctv,
                        iwt[:, 64 * k:64 * (k + 1)],
                        channels=128, num_elems=H * W, d=8,
                        num_idxs=PPOS)
                    # pre-emit next iteration's replication so PE keeps
                    # ahead of the serial gather stream
                    if not (ph == NPH - 1 and k == KK - 1):
                        nk = (k + 1) % KK
                        nph = ph + 1 if nk == 0 else ph
                        wre_next = emit_rep(nph, nk)
                    PA = prod_p.tile([128, PPOS, 4], BF16, tag="pa")
                    PB = prod_p.tile([128, PPOS, 4], BF16, tag="pb")
                    valA = val_p.tile([128, PPOS], BF16, tag="valA")
                    valB = val_p.tile([128, PPOS], BF16, tag="valB")
                    wr4 = wre[:].rearrange("p (n e) -> p n e", e=4)
                    v.tensor_tensor(PA[:], g_[:, :, 0:4], wr4, OP.mult)
                    v.tensor_tensor(PB[:], g_[:, :, 4:8], wr4, OP.mult)
                    v.tensor_tensor(PA[:, :, 0:2], PA[:, :, 0:2],
                                    PA[:, :, 2:4], OP.add)
                    v.tensor_tensor(valA[:], PA[:, :, 0], PA[:, :, 1],
                                    OP.add)
                    v.tensor_tensor(PB[:, :, 0:2], PB[:, :, 0:2],
                                    PB[:, :, 2:4], OP.add)
                    v.tensor_tensor(valB[:], PB[:, :, 0], PB[:, :, 1],
                                    OP.add)
                    first = (k == 0)
                    last = (k == KK - 1)
                    for h_, vv in ((0, valA), (1, valB)):
                        for fc_ in range(2):
                            for ns in range(2):
                                pe.matmul(
                                    dps[2 * fc_ + ns][:],
                                    dwv[:, k, h_,
                                        128 * fc_:128 * (fc_ + 1)],
                                    vv[:, 512 * ns:512 * (ns + 1)],
                                    start=(first and h_ == 0),
                                    stop=(last and h_ == 1))
                for fc_ in range(2):
                    for ns in range(2):
                        s.activation(
                            AL[fc_][:, p0 + 512 * ns:p0 + 512 * (ns + 1)],
                            dps[2 * fc_ + ns][:], AF.Relu,
                            bias=dcnbt[:, fc_:fc_ + 1])

        # ---- final add + transpose + out
        with tc.tile_pool(name="fin_ps", bufs=2, space="PSUM") as fin_ps, \
             tc.tile_pool(name="fin_sb", bufs=2) as fin_sb:
            for fc_ in range(2):
                v.tensor_tensor(AL[fc_][:], AL[fc_][:],
                                fcal[fc_][:, W:W + POS], OP.add)
            for i in range(16):
                ot = fin_sb.tile([128, C], FP32, tag="ot")
                for fc_ in range(2):
                    ps = fin_ps.tile([128, 128], FP32, tag="fin")
                    pe.transpose(ps[:], AL[fc_][:, 128 * i:128 * (i + 1)],
                                 idt[:])
                    s.copy(ot[:, 128 * fc_:128 * (fc_ + 1)], ps[:])
                nc.sync.dma_start(out_d[128 * i:128 * (i + 1), :], ot[:])

    nc.compile()
    return nc


TileCtx = tile.TileContext
_NC_CACHE = None


def _get_nc():
    global _NC_CACHE
    if _NC_CACHE is None:
        _NC_CACHE = _build_nc()
    return _NC_CACHE


def kernel(fine, coarse, attend_w, select_w, offset_w, om_w, om_b, dcn_w,
           dcn_b, _trace=False, _trace_kwargs=None):
    wd = _prep_weights(np.asarray(attend_w), np.asarray(select_w),
                       np.asarray(offset_w), np.asarray(om_w),
                       np.asarray(om_b), np.asarray(dcn_w), np.asarray(dcn_b))
    in_maps = [_core_inputs(c, np.asarray(fine), np.asarray(coarse), wd)
               for c in range(NCORES)]
    nc = _get_nc()
    kw = {}
    if _trace:
        import concourse.bass_utils as _bu
        _bu.upload_artifacts = lambda d: d
        kw = dict(trace=True, trace_kwargs=_trace_kwargs or {})
    res = run_bass_kernel_spmd(nc, in_maps, list(range(NCORES)), **kw)
    out = np.zeros((B, H, W, C), np.float32)
    for c in range(NCORES):
        b, half = c // 2, c % 2
        out[b, 32 * half:32 * half + 32] = res.results[c]["out"].reshape(32, W, C)
    if _trace:
        kernel._last_exec_ns = res.exec_time_ns
    return out
